# revision 5
# baseline (speedup 1.0000x reference)
"""AtomGIN (3-layer GIN message passing) on 8 Trainium2 NeuronCores.

Strategy (graph/data parallel, dst-partitioned):
  - Nodes are split evenly across 8 cores (core c owns rows [c*6250, (c+1)*6250)).
  - Each core processes the edges whose DESTINATION is local. Message
    aggregation (segment_sum over incoming edges) is computed as a sequence of
    PE matmuls: for each window of 128 destination nodes, PSUM accumulates
    msg_tile.T @ onehot(dst_offset) over that window's edge tiles.
  - h (node features) is replicated in every core's HBM via AllGather after
    each layer; per-edge source rows are fetched with the GPSIMD dma_gather
    instruction (int16 indices; h is addressed as two halves so indices fit
    in 15 bits). Gathers rotate over the 4 SWDGE queues so descriptor
    generation runs on all four Q7 core pairs concurrently.
  - The destination one-hot matrices are STATIC (edge schedule only), so they
    are precomputed on the host, shipped to HBM, and streamed into SBUF with
    cheap contiguous HWDGE DMAs (no on-device is_equal builds).
  - Self-loop messages are NOT gathered: the feature-major copy of h (hTb,
    kept in SBUF for the transpose path anyway) is added directly into the
    aggregate during the PSUM->SBUF copy. The self-loop edge-embedding term
    stays in the count matmul (code 12 counts).
  - The edge-embedding term is aggregated analytically: for each destination
    node, sum_e eemb[code_e] = table15.T @ counts[:, dst], folded into the
    same PSUM accumulation. The initial atom embedding is likewise a one-hot
    matmul t9.T @ acnt.
  - The per-layer MLP runs in feature-major (transposed) layout so W1/W2 act
    as stationary matmul operands with no activation transposes (W1 stage in
    bf16, W2 stage in f32); BatchNorm statistics are accumulated per
    destination tile (overlapped with the aggregation phase) and combined
    across cores with a tiny AllReduce.
"""

import numpy as np

N = 50000
E = 500000
D = 128
L = 3
BN_EPS = 1e-5
P = 128
NCORES = 8
NPC = N // NCORES            # nodes per core
NT = (NPC + P - 1) // P      # node tiles per core
NPAD = NT * P
VHALF = 32768                # h table split (int16 index headroom)
CH_TILES = 24                # edge tiles per dma_gather call (3072 indices)
NQ = 4                       # SWDGE queues to round-robin


def _wrap_idx_cols(idx2d):
    """[rows] int -> dma_gather wrapped layout [128, rows//16] int16.

    Index i lives at (partition i%16, col i//16), replicated 8x down the
    partition axis (one copy per Q7 core).
    """
    n = idx2d.shape[0]
    w = idx2d.reshape(n // 16, 16).T.astype(np.int16)
    return np.tile(w, (8, 1))


def _preprocess(x, edge_index, edge_attr):
    """Host-side integer preprocessing. Returns (schedule, per-core arrays)."""
    x = np.asarray(x)
    ei = np.asarray(edge_index)
    ea = np.asarray(edge_attr)

    code_a = (x[:, 0] * 3 + x[:, 1]).astype(np.int64)          # [N] in 0..8
    src = ei[0].astype(np.int64)
    dst = ei[1].astype(np.int64)
    ecode = (ea[:, 0] * 3 + ea[:, 1]).astype(np.int64)

    core = dst // NPC
    dst_local = dst - core * NPC
    nt_of_edge = dst_local // P
    # permuted h row for a source node: shards are stored partition-major
    # ([p, t, d] flat), so node (c, i) lands at row c*NPAD + (i%P)*NT + i//P
    s_core = src // NPC
    s_loc = src - s_core * NPC
    grow = s_core * NPAD + (s_loc % P) * NT + s_loc // P
    is_hi = grow >= VHALF

    # --- per (core, node-tile) lo/hi counts -> shared edge-tile schedule ---
    cnt_lo = np.zeros((NCORES, NT), np.int64)
    cnt_hi = np.zeros((NCORES, NT), np.int64)
    np.add.at(cnt_lo, (core[~is_hi], nt_of_edge[~is_hi]), 1)
    np.add.at(cnt_hi, (core[is_hi], nt_of_edge[is_hi]), 1)
    k_lo = np.maximum(1, ((cnt_lo + P - 1) // P).max(axis=0))   # [NT]
    k_hi = np.maximum(1, ((cnt_hi + P - 1) // P).max(axis=0))
    et_lo = int(k_lo.sum())
    et_hi = int(k_hi.sum())
    start_lo = np.zeros(NT + 1, np.int64)
    start_lo[1:] = np.cumsum(k_lo)
    start_hi = np.zeros(NT + 1, np.int64)
    start_hi[1:] = np.cumsum(k_hi)

    gidx_lo = np.zeros((NCORES, et_lo * P), np.int64)
    gidx_hi = np.zeros((NCORES, et_hi * P), np.int64)
    # one-hot tiles [cores, P, et*P]: slot s of tile j maps to dst column
    ohm_lo = np.zeros((NCORES, P, et_lo * P), np.float32)
    ohm_hi = np.zeros((NCORES, P, et_hi * P), np.float32)

    for c in range(NCORES):
        m = core == c
        s_c, dl_c, hi_c = grow[m], dst_local[m], is_hi[m]
        for hi_flag, gidx, ohm, starts in (
            (False, gidx_lo, ohm_lo, start_lo),
            (True, gidx_hi, ohm_hi, start_hi),
        ):
            sel = hi_c == hi_flag
            s_s = s_c[sel] - (VHALF if hi_flag else 0)
            dl_s = dl_c[sel]
            order = np.argsort(dl_s, kind="stable")
            s_s, dl_s = s_s[order], dl_s[order]
            nt_s = dl_s // P
            seg_first = np.searchsorted(nt_s, np.arange(NT))
            rank = np.arange(len(dl_s)) - seg_first[nt_s]
            tile_of = starts[nt_s] + rank // P
            slot = tile_of * P + rank % P
            gidx[c, slot] = s_s
            ohm[c, rank % P, tile_of * P + (dl_s - nt_s * P)] = 1.0

    # edge-code count matrix [cores, 16, NPAD] (codes 0..14; row 15 zero-pad)
    # self-loop edges (code 4*3+0=12, one per node) contribute only here.
    cnt = np.zeros((NCORES, 16, NPAD), np.float32)
    np.add.at(cnt, (core, ecode, dst_local), 1.0)
    loop_l = np.arange(N, dtype=np.int64) % NPC
    cnt[np.arange(N) // NPC, 12, loop_l] += 1.0

    # atom-code one-hot [cores, 16, NPAD]
    acnt = np.zeros((NCORES, 16, NPAD), np.float32)
    acnt[np.arange(N) // NPC, code_a, loop_l] = 1.0

    sched = dict(k_lo=k_lo.tolist(), k_hi=k_hi.tolist(),
                 et_lo=et_lo, et_hi=et_hi,
                 start_lo=start_lo.tolist(), start_hi=start_hi.tolist())
    arrays = dict(gidx_lo=gidx_lo, gidx_hi=gidx_hi,
                  ohm_lo=ohm_lo, ohm_hi=ohm_hi,
                  cnt=cnt, acnt=acnt)
    return sched, arrays


def _build(sched):
    """Build the SPMD Bacc graph (one graph, run on all 8 cores)."""
    import concourse.bacc as bacc
    import concourse.mybir as mybir
    from concourse.tile import TileContext

    f32 = mybir.dt.float32
    bf16 = mybir.dt.bfloat16
    i16 = mybir.dt.int16
    ACT = mybir.ActivationFunctionType
    ALU = mybir.AluOpType

    k_lo, k_hi = sched["k_lo"], sched["k_hi"]
    et_lo, et_hi = sched["et_lo"], sched["et_hi"]
    start_lo, start_hi = sched["start_lo"], sched["start_hi"]

    nc = bacc.Bacc("TRN2", target_bir_lowering=False, debug=False,
                   num_devices=NCORES, num_swdge_queues=NQ)

    def inp(name, shape, dt):
        return nc.declare_dram_parameter(name, list(shape), dt, isOutput=False)

    gidx_lo = inp("gidx_lo", [P, et_lo * 8], i16)
    gidx_hi = inp("gidx_hi", [P, et_hi * 8], i16)
    ohm_lo = inp("ohm_lo", [P, et_lo * P], bf16)
    ohm_hi = inp("ohm_hi", [P, et_hi * P], bf16)
    cnt_in = inp("cnt", [16, NPAD], f32)
    acnt_in = inp("acnt", [16, NPAD], bf16)
    prm_in = inp("prmT", [P, 8 * L], f32)      # cols: l*8 + (b1a,b1b,b2,gamma,beta,0,0,0)
    w1_in = inp("w1", [L, D, 2 * D], bf16)
    w2_in = inp("w2", [L, 2 * D, D], f32)
    t9_in = inp("t9", [16, D], bf16)
    t15_in = inp("t15", [L, 16, D], f32)
    idf_in = inp("identf", [P, P], f32)
    idb_in = inp("identb", [P, P], bf16)
    out_ext = nc.declare_dram_parameter("out", [NPC, D], f32, isOutput=True)

    NROWS = NCORES * NPAD
    h_loc = nc.dram_tensor("h_loc", [NROWS, D], bf16, addr_space="Shared")
    h_shard = nc.dram_tensor("h_shard", [NPAD, D], bf16)
    st_loc = nc.dram_tensor("st_loc", [P, 2], f32)
    st_glob = nc.dram_tensor("st_glob", [P, 2], f32, addr_space="Shared")
    RG = [list(range(NCORES))]

    nfull = NPC // P
    rem = NPC - nfull * P
    inv_n = 1.0 / float(N)

    with TileContext(nc) as tc:
        with tc.tile_pool(name="cst", bufs=1) as cp, \
             tc.tile_pool(name="big", bufs=1) as bp, \
             tc.tile_pool(name="nrm", bufs=1) as np_pool, \
             tc.tile_pool(name="wgt", bufs=2) as wp, \
             tc.tile_pool(name="msg", bufs=6) as mp, \
             tc.tile_pool(name="ohp", bufs=4) as op_, \
             tc.tile_pool(name="act", bufs=4) as ap_, \
             tc.tile_pool(name="sml", bufs=1) as sp:

            # ---- persistent constants / inputs in SBUF ----
            def load(pool, shape, dt, src, tag):
                t = pool.tile(list(shape), dt, tag=tag)
                nc.sync.dma_start(out=t[:], in_=src)
                return t

            gi_lo = load(cp, [P, et_lo * 8], i16, gidx_lo[:, :], "gi_lo")
            gi_hi = load(cp, [P, et_hi * 8], i16, gidx_hi[:, :], "gi_hi")

            cnt_sb = load(cp, [16, NPAD], f32, cnt_in[:, :], "cnt")
            acnt_sb = load(cp, [16, NPAD], bf16, acnt_in[:, :], "acnt")
            t9_sb = load(cp, [16, D], bf16, t9_in[:, :], "t9")
            prm_sb = load(cp, [P, 8 * L], f32, prm_in[:, :], "prm")
            idf_sb = load(cp, [P, P], f32, idf_in[:, :], "idf")
            idb_sb = load(cp, [P, P], bf16, idb_in[:, :], "idb")

            # ---- big working buffers ----
            outT = bp.tile([P, NPAD], f32)          # post-MLP, feature-major
            h_nm = bp.tile([P, NT * D], bf16)       # node-major h staging
            hTb = np_pool.tile([P, NPAD], bf16)     # feature-major h (layer input)

            # small stat tiles
            sumc_sb = sp.tile([P, NT], f32)
            sqc_sb = sp.tile([P, NT], f32)
            stats_sb = sp.tile([P, 2], f32)
            gstats_sb = sp.tile([P, 2], f32)
            mean_c = sp.tile([P, 1], f32)
            ex2_c = sp.tile([P, 1], f32)
            msq_c = sp.tile([P, 1], f32)
            var_c = sp.tile([P, 1], f32)
            sd_c = sp.tile([P, 1], f32)
            rstd_c = sp.tile([P, 1], f32)
            k_c = sp.tile([P, 1], f32)
            tmp_c = sp.tile([P, 1], f32)
            c_c = sp.tile([P, 1], f32)

            def store_shard(src_sb, dst_dram):
                """node-major SBUF [128, NT*D] -> DRAM [NPC, D] (strided)."""
                if nfull:
                    nc.sync.dma_start(
                        out=dst_dram[0:nfull * P, :].rearrange("(t p) d -> p t d", p=P),
                        in_=src_sb[:, 0:nfull * D].rearrange("p (t d) -> p t d", d=D))
                if rem:
                    nc.sync.dma_start(
                        out=dst_dram[nfull * P:NPC, :],
                        in_=src_sb[0:rem, nfull * D:(nfull + 1) * D])

            def store_shard_pm(src_sb):
                """SBUF [128, NT*D] -> h_shard [NPAD, D] partition-major, contiguous."""
                nc.sync.dma_start(
                    out=h_shard.ap().rearrange("(p t) d -> p t d", t=NT),
                    in_=src_sb[:].rearrange("p (t d) -> p t d", d=D))

            # ================= embedding phase =================
            # h0 feature-major = t9.T @ acnt (per destination tile), then
            # transpose to node-major for the gather table.
            with tc.tile_pool(name="pse", bufs=2, space="PSUM") as pe_, \
                 tc.tile_pool(name="pst0", bufs=2, space="PSUM") as pt0:
                for nt in range(NT):
                    ps = pe_.tile([P, P], f32, tag="pse")
                    nc.tensor.matmul(out=ps[:], lhsT=t9_sb[:],
                                     rhs=acnt_sb[:, nt * P:(nt + 1) * P],
                                     start=True, stop=True)
                    nc.vector.tensor_scalar_add(
                        hTb[:, nt * P:(nt + 1) * P], ps[:], 0.0)
                    pst = pt0.tile([P, P], bf16, tag="pst0")
                    nc.tensor.transpose(
                        out=pst[:], in_=hTb[:, nt * P:(nt + 1) * P],
                        identity=idb_sb[:])
                    nc.scalar.activation(h_nm[:, nt * D:(nt + 1) * D],
                                         pst[:], ACT.Copy)
            store_shard_pm(h_nm)
            nc.gpsimd.collective_compute(
                "AllGather", mybir.AluOpType.bypass, replica_groups=RG,
                ins=[h_shard.ap().opt()], outs=[h_loc.ap().opt()])

            # ================= layers =================
            for l in range(L):
                w1_sb = load(wp, [D, 2 * D], bf16, w1_in[l, :, :], tag="w1")
                w2a_sb = load(wp, [D, D], f32, w2_in[l, 0:D, :], tag="w2a")
                w2b_sb = load(wp, [D, D], f32, w2_in[l, D:2 * D, :], tag="w2b")
                t15_sb = load(wp, [16, D], f32, t15_in[l, :, :], tag="t15")

                b1a = prm_sb[:, l * 8 + 0:l * 8 + 1]
                b1b = prm_sb[:, l * 8 + 1:l * 8 + 2]
                b2 = prm_sb[:, l * 8 + 2:l * 8 + 3]
                gam = prm_sb[:, l * 8 + 3:l * 8 + 4]
                bet = prm_sb[:, l * 8 + 4:l * 8 + 5]

                # lazy chunked gathers + one-hot loads per stream
                chunks = {"lo": {}, "hi": {}}
                stream_cfg = {
                    "lo": (gi_lo, ohm_lo, et_lo, 0),
                    "hi": (gi_hi, ohm_hi, et_hi, VHALF),
                }

                def msg_slice(s, pos):
                    gi, ohm_d, et_s, row0 = stream_cfg[s]
                    ci = pos // CH_TILES
                    if ci not in chunks[s]:
                        ntile = min(CH_TILES, et_s - ci * CH_TILES)
                        nidx = ntile * P
                        g = mp.tile([P, CH_TILES * D], bf16, tag="msg")
                        nc.gpsimd.dma_gather(
                            out_ap=g[:, 0:ntile * D].rearrange("p (t e) -> p t e", e=D),
                            in_ap=h_loc[row0:NROWS, :] if row0 else h_loc[0:VHALF, :],
                            idxs_ap=gi[:, ci * CH_TILES * 8: ci * CH_TILES * 8 + nidx // 16],
                            num_idxs=nidx, num_idxs_reg=nidx, elem_size=D,
                            single_packet=False, queue_num=0)
                        ohc = op_.tile([P, CH_TILES * P], bf16, tag="ohc")
                        t0 = ci * CH_TILES
                        nc.sync.dma_start(
                            out=ohc[:, 0:ntile * P],
                            in_=ohm_d[:, t0 * P:(t0 + ntile) * P])
                        chunks[s][ci] = (g, ohc)
                    g, ohc = chunks[s][ci]
                    j = pos % CH_TILES
                    return g[:, j * D:(j + 1) * D], ohc[:, j * P:(j + 1) * P]

                with tc.tile_pool(name="psa", bufs=2, space="PSUM") as pa, \
                     tc.tile_pool(name="psh", bufs=2, space="PSUM") as ph, \
                     tc.tile_pool(name="pso", bufs=2, space="PSUM") as po:
                    for nt in range(NT):
                        psa = pa.tile([P, P], f32, tag="psa")
                        first = True
                        for s, karr, starts in (
                            ("lo", k_lo, start_lo),
                            ("hi", k_hi, start_hi),
                        ):
                            for jj in range(karr[nt]):
                                pos = starts[nt] + jj
                                m, oh = msg_slice(s, pos)
                                nc.tensor.matmul(out=psa[:], lhsT=m, rhs=oh,
                                                 start=first, stop=False)
                                first = False
                        nc.tensor.matmul(out=psa[:], lhsT=t15_sb[:],
                                         rhs=cnt_sb[:, nt * P:(nt + 1) * P],
                                         start=first, stop=True)

                        # aggr = psa + h_v (self-loop message), in bf16
                        aggr_b = ap_.tile([P, P], bf16, tag="aggr")
                        nc.vector.tensor_tensor(
                            out=aggr_b[:], in0=psa[:],
                            in1=hTb[:, nt * P:(nt + 1) * P], op=ALU.add)

                        psh1 = ph.tile([P, P], f32, tag="psh")
                        nc.tensor.matmul(out=psh1[:], lhsT=w1_sb[:, 0:D],
                                         rhs=aggr_b[:], start=True, stop=True)
                        hidA = ap_.tile([P, P], f32, tag="hidA")
                        nc.scalar.activation(hidA[:], psh1[:], ACT.Relu, bias=b1a)
                        psh2 = ph.tile([P, P], f32, tag="psh")
                        nc.tensor.matmul(out=psh2[:], lhsT=w1_sb[:, D:2 * D],
                                         rhs=aggr_b[:], start=True, stop=True)
                        hidB = ap_.tile([P, P], f32, tag="hidB")
                        nc.scalar.activation(hidB[:], psh2[:], ACT.Relu, bias=b1b)

                        pso1 = po.tile([P, P], f32, tag="pso")
                        nc.tensor.matmul(out=pso1[:], lhsT=w2a_sb[:], rhs=hidA[:],
                                         start=True, stop=False)
                        nc.tensor.matmul(out=pso1[:], lhsT=w2b_sb[:], rhs=hidB[:],
                                         start=False, stop=True)
                        nc.vector.tensor_scalar_add(
                            outT[:, nt * P:(nt + 1) * P], pso1[:], b2)

                        # per-tile BN stat partials (valid node columns only)
                        c0 = nt * P
                        c1 = min((nt + 1) * P, NPC)
                        if c1 > c0:
                            nc.vector.tensor_reduce(
                                out=sumc_sb[:, nt:nt + 1],
                                in_=outT[:, c0:c1],
                                axis=mybir.AxisListType.X, op=ALU.add)
                            sq_scr = ap_.tile([P, P], f32, tag="sqscr")
                            nc.scalar.activation(
                                sq_scr[:, 0:c1 - c0], outT[:, c0:c1], ACT.Square,
                                accum_out=sqc_sb[:, nt:nt + 1])

                # ---- batch-norm statistics: combine tile partials ----
                nc.vector.tensor_reduce(
                    out=stats_sb[:, 0:1], in_=sumc_sb[:],
                    axis=mybir.AxisListType.X, op=ALU.add)
                nc.vector.tensor_reduce(
                    out=stats_sb[:, 1:2], in_=sqc_sb[:],
                    axis=mybir.AxisListType.X, op=ALU.add)
                nc.sync.dma_start(out=st_loc[:, :], in_=stats_sb[:])
                nc.gpsimd.collective_compute(
                    "AllReduce", ALU.add, replica_groups=RG,
                    ins=[st_loc.ap().opt()], outs=[st_glob.ap().opt()])
                nc.sync.dma_start(out=gstats_sb[:], in_=st_glob[:, :])

                nc.vector.tensor_scalar_mul(mean_c[:], gstats_sb[:, 0:1], inv_n)
                nc.vector.tensor_scalar_mul(ex2_c[:], gstats_sb[:, 1:2], inv_n)
                nc.scalar.activation(msq_c[:], mean_c[:], ACT.Square)
                nc.vector.tensor_tensor(var_c[:], ex2_c[:], msq_c[:], op=ALU.subtract)
                nc.vector.tensor_scalar_add(var_c[:], var_c[:], BN_EPS)
                nc.scalar.activation(sd_c[:], var_c[:], ACT.Sqrt)
                nc.vector.reciprocal(rstd_c[:], sd_c[:])
                nc.vector.tensor_tensor(k_c[:], gam, rstd_c[:], op=ALU.mult)
                nc.vector.tensor_tensor(tmp_c[:], mean_c[:], k_c[:], op=ALU.mult)
                nc.vector.tensor_tensor(c_c[:], bet, tmp_c[:], op=ALU.subtract)

                with tc.tile_pool(name="pst", bufs=2, space="PSUM") as pt:
                    if l < L - 1:
                        # h = relu(out*k + c) in bf16, transpose to node-major
                        nc.scalar.activation(hTb[:], outT[:], ACT.Relu,
                                             bias=c_c[:, 0:1], scale=k_c[:, 0:1])
                        for nt in range(NT):
                            pst = pt.tile([P, P], bf16, tag="pst")
                            nc.tensor.transpose(
                                out=pst[:], in_=hTb[:, nt * P:(nt + 1) * P],
                                identity=idb_sb[:])
                            nc.scalar.activation(h_nm[:, nt * D:(nt + 1) * D],
                                                 pst[:], ACT.Copy)
                        store_shard_pm(h_nm)
                        nc.gpsimd.collective_compute(
                            "AllGather", mybir.AluOpType.bypass, replica_groups=RG,
                            ins=[h_shard.ap().opt()], outs=[h_loc.ap().opt()])
                    else:
                        # final: out*k + c in place, transpose, store per tile
                        nc.vector.tensor_scalar(
                            out=outT[:], in0=outT[:],
                            scalar1=k_c[:, 0:1], scalar2=c_c[:, 0:1],
                            op0=ALU.mult, op1=ALU.add)
                        for nt in range(NT):
                            pst = pt.tile([P, P], f32, tag="pstf")
                            nc.tensor.transpose(
                                out=pst[:], in_=outT[:, nt * P:(nt + 1) * P],
                                identity=idf_sb[:])
                            o_t = ap_.tile([P, P], f32, tag="otile")
                            nc.scalar.activation(o_t[:], pst[:], ACT.Copy)
                            r0 = nt * P
                            r1 = min((nt + 1) * P, NPC)
                            if r1 > r0:
                                nc.sync.dma_start(
                                    out=out_ext[r0:r1, :],
                                    in_=o_t[0:r1 - r0, :])

    # Align each gather's SWDGE queue with the DMASW semaphore lane Tile
    # assigned it (lane k <-> queue k % NQ), so no semaphore is shared by
    # two queues (completion order within a lane must match issue order).
    from concourse.tile_scheduler import PROC_NAME_TO_IDX
    dmasw0 = PROC_NAME_TO_IDX["DMASW0"]
    for inst in nc.inst_map.values():
        if isinstance(inst, mybir.InstDMAGatherAnt):
            proc = inst.bass_scheduled_proc
            assert proc is not None and dmasw0 <= proc < dmasw0 + 8, (
                f"gather {inst.name} not on a DMASW lane: {proc}")
            inst.queue_num = (proc - dmasw0) % NQ

    nc.compile()
    return nc


_CACHE = {}


def _make_in_maps(arr, atom_emb0, atom_emb1, edge_emb0, edge_emb1,
                  W1, b1, W2, b2, gamma, beta):
    import ml_dtypes
    # ---- parameter tables (host float prep limited to tiny tables) ----
    ae0 = np.asarray(atom_emb0, np.float32)
    ae1 = np.asarray(atom_emb1, np.float32)
    ee0 = np.asarray(edge_emb0, np.float32)
    ee1 = np.asarray(edge_emb1, np.float32)
    t9 = np.zeros((16, D), np.float32)
    t9[:9] = (ae0[:3, None, :] + ae1[None, :3, :]).reshape(9, D)
    t15 = np.zeros((L, 16, D), np.float32)
    for l in range(L):
        t15[l, :15] = (ee0[l][:, None, :] + ee1[l][None, :, :]).reshape(15, D)

    W1 = np.asarray(W1, np.float32)
    W2 = np.asarray(W2, np.float32)
    b1 = np.asarray(b1, np.float32)
    b2 = np.asarray(b2, np.float32)
    gamma = np.asarray(gamma, np.float32)
    beta = np.asarray(beta, np.float32)
    prmT = np.zeros((P, 8 * L), np.float32)
    for l in range(L):
        prmT[:, l * 8 + 0] = b1[l, 0:D]
        prmT[:, l * 8 + 1] = b1[l, D:2 * D]
        prmT[:, l * 8 + 2] = b2[l]
        prmT[:, l * 8 + 3] = gamma[l]
        prmT[:, l * 8 + 4] = beta[l]

    ident = np.eye(P, dtype=np.float32)
    bf = ml_dtypes.bfloat16

    in_maps = []
    for c in range(NCORES):
        in_maps.append({
            "gidx_lo": _wrap_idx_cols(arr["gidx_lo"][c]),
            "gidx_hi": _wrap_idx_cols(arr["gidx_hi"][c]),
            "ohm_lo": arr["ohm_lo"][c].astype(bf),
            "ohm_hi": arr["ohm_hi"][c].astype(bf),
            "cnt": arr["cnt"][c],
            "acnt": arr["acnt"][c].astype(bf),
            "prmT": prmT,
            "w1": W1.astype(bf),
            "w2": W2,
            "t9": t9.astype(bf),
            "t15": t15,
            "identf": ident,
            "identb": ident.astype(bf),
        })
    return in_maps


def kernel(x, edge_index, edge_attr, atom_emb0, atom_emb1,
           edge_emb0, edge_emb1, W1, b1, W2, b2, gamma, beta):
    from concourse.bass_utils import run_bass_kernel_spmd

    sched, arr = _preprocess(x, edge_index, edge_attr)
    key = (tuple(sched["k_lo"]), tuple(sched["k_hi"]))
    if key not in _CACHE:
        _CACHE[key] = _build(sched)
    nc = _CACHE[key]

    in_maps = _make_in_maps(arr, atom_emb0, atom_emb1, edge_emb0, edge_emb1,
                            W1, b1, W2, b2, gamma, beta)
    res = run_bass_kernel_spmd(nc, in_maps, core_ids=list(range(NCORES)))
    out = np.concatenate([res.results[c]["out"] for c in range(NCORES)], axis=0)
    return out.astype(np.float32)


# revision 6
# speedup vs baseline: 1.0844x; 1.0844x over previous
"""AtomGIN (3-layer GIN message passing) on 8 Trainium2 NeuronCores.

Strategy (graph/data parallel, dst-partitioned):
  - Nodes are split evenly across 8 cores (core c owns rows [c*6250, (c+1)*6250)).
  - Each core processes the edges whose DESTINATION is local. Message
    aggregation (segment_sum over incoming edges) is computed as a sequence of
    PE matmuls: for each window of 128 destination nodes, PSUM accumulates
    msg_tile.T @ onehot(dst_offset) over that window's edge tiles.
  - h (node features) is replicated in every core's HBM via AllGather after
    each layer; per-edge source rows are fetched with the GPSIMD dma_gather
    instruction (int16 indices; h is addressed as two halves so indices fit
    in 15 bits). Gathers rotate over the 4 SWDGE queues so descriptor
    generation runs on all four Q7 core pairs concurrently.
  - The destination one-hot matrices are STATIC (edge schedule only), so they
    are precomputed on the host, shipped to HBM, and streamed into SBUF with
    cheap contiguous HWDGE DMAs (no on-device is_equal builds).
  - Self-loop messages are NOT gathered: the feature-major copy of h (hTb,
    kept in SBUF for the transpose path anyway) is added directly into the
    aggregate during the PSUM->SBUF copy. The self-loop edge-embedding term
    stays in the count matmul (code 12 counts).
  - The edge-embedding term is aggregated analytically: for each destination
    node, sum_e eemb[code_e] = table15.T @ counts[:, dst], folded into the
    same PSUM accumulation. The initial atom embedding is likewise a one-hot
    matmul t9.T @ acnt.
  - The per-layer MLP runs in feature-major (transposed) layout so W1/W2 act
    as stationary matmul operands with no activation transposes (W1 stage in
    bf16, W2 stage in f32); BatchNorm statistics are accumulated per
    destination tile (overlapped with the aggregation phase) and combined
    across cores with a tiny AllReduce.
"""

import numpy as np

N = 50000
E = 500000
D = 128
L = 3
BN_EPS = 1e-5
P = 128
NCORES = 8
NPC = N // NCORES            # nodes per core
NT = (NPC + P - 1) // P      # node tiles per core
NPAD = NT * P
VHALF = 32768                # h table split (int16 index headroom)
CH_TILES = 24                # edge tiles per dma_gather call (3072 indices)
NQ = 4                       # SWDGE queues to round-robin


def _wrap_idx_cols(idx2d):
    """[rows] int -> dma_gather wrapped layout [128, rows//16] int16.

    Index i lives at (partition i%16, col i//16), replicated 8x down the
    partition axis (one copy per Q7 core).
    """
    n = idx2d.shape[0]
    w = idx2d.reshape(n // 16, 16).T.astype(np.int16)
    return np.tile(w, (8, 1))


def _preprocess(x, edge_index, edge_attr):
    """Host-side integer preprocessing. Returns (schedule, per-core arrays)."""
    x = np.asarray(x)
    ei = np.asarray(edge_index)
    ea = np.asarray(edge_attr)

    code_a = (x[:, 0] * 3 + x[:, 1]).astype(np.int64)          # [N] in 0..8
    src = ei[0].astype(np.int64)
    dst = ei[1].astype(np.int64)
    ecode = (ea[:, 0] * 3 + ea[:, 1]).astype(np.int64)

    core = dst // NPC
    dst_local = dst - core * NPC
    nt_of_edge = dst_local // P
    # permuted h row for a source node: shards are stored partition-major
    # ([p, t, d] flat), so node (c, i) lands at row c*NPAD + (i%P)*NT + i//P
    s_core = src // NPC
    s_loc = src - s_core * NPC
    grow = s_core * NPAD + (s_loc % P) * NT + s_loc // P
    is_hi = grow >= VHALF

    # --- per (core, node-tile) lo/hi counts -> shared edge-tile schedule ---
    cnt_lo = np.zeros((NCORES, NT), np.int64)
    cnt_hi = np.zeros((NCORES, NT), np.int64)
    np.add.at(cnt_lo, (core[~is_hi], nt_of_edge[~is_hi]), 1)
    np.add.at(cnt_hi, (core[is_hi], nt_of_edge[is_hi]), 1)
    k_lo = np.maximum(1, ((cnt_lo + P - 1) // P).max(axis=0))   # [NT]
    k_hi = np.maximum(1, ((cnt_hi + P - 1) // P).max(axis=0))
    et_lo = int(k_lo.sum())
    et_hi = int(k_hi.sum())
    start_lo = np.zeros(NT + 1, np.int64)
    start_lo[1:] = np.cumsum(k_lo)
    start_hi = np.zeros(NT + 1, np.int64)
    start_hi[1:] = np.cumsum(k_hi)

    gidx_lo = np.zeros((NCORES, et_lo * P), np.int64)
    gidx_hi = np.zeros((NCORES, et_hi * P), np.int64)
    # one-hot tiles [cores, P, et*P]: slot s of tile j maps to dst column
    ohm_lo = np.zeros((NCORES, P, et_lo * P), np.float32)
    ohm_hi = np.zeros((NCORES, P, et_hi * P), np.float32)

    for c in range(NCORES):
        m = core == c
        s_c, dl_c, hi_c = grow[m], dst_local[m], is_hi[m]
        for hi_flag, gidx, ohm, starts in (
            (False, gidx_lo, ohm_lo, start_lo),
            (True, gidx_hi, ohm_hi, start_hi),
        ):
            sel = hi_c == hi_flag
            s_s = s_c[sel] - (VHALF if hi_flag else 0)
            dl_s = dl_c[sel]
            order = np.argsort(dl_s, kind="stable")
            s_s, dl_s = s_s[order], dl_s[order]
            nt_s = dl_s // P
            seg_first = np.searchsorted(nt_s, np.arange(NT))
            rank = np.arange(len(dl_s)) - seg_first[nt_s]
            tile_of = starts[nt_s] + rank // P
            slot = tile_of * P + rank % P
            gidx[c, slot] = s_s
            ohm[c, rank % P, tile_of * P + (dl_s - nt_s * P)] = 1.0

    # edge-code count matrix [cores, 16, NPAD] (codes 0..14; row 15 zero-pad)
    # self-loop edges (code 4*3+0=12, one per node) contribute only here.
    cnt = np.zeros((NCORES, 16, NPAD), np.float32)
    np.add.at(cnt, (core, ecode, dst_local), 1.0)
    loop_l = np.arange(N, dtype=np.int64) % NPC
    cnt[np.arange(N) // NPC, 12, loop_l] += 1.0

    # atom-code one-hot [cores, 16, NPAD]
    acnt = np.zeros((NCORES, 16, NPAD), np.float32)
    acnt[np.arange(N) // NPC, code_a, loop_l] = 1.0

    sched = dict(k_lo=k_lo.tolist(), k_hi=k_hi.tolist(),
                 et_lo=et_lo, et_hi=et_hi,
                 start_lo=start_lo.tolist(), start_hi=start_hi.tolist())
    arrays = dict(gidx_lo=gidx_lo, gidx_hi=gidx_hi,
                  ohm_lo=ohm_lo, ohm_hi=ohm_hi,
                  cnt=cnt, acnt=acnt)
    return sched, arrays


def _build(sched):
    """Build the SPMD Bacc graph (one graph, run on all 8 cores)."""
    import concourse.bacc as bacc
    import concourse.mybir as mybir
    from concourse.tile import TileContext

    f32 = mybir.dt.float32
    bf16 = mybir.dt.bfloat16
    i16 = mybir.dt.int16
    ACT = mybir.ActivationFunctionType
    ALU = mybir.AluOpType

    k_lo, k_hi = sched["k_lo"], sched["k_hi"]
    et_lo, et_hi = sched["et_lo"], sched["et_hi"]
    start_lo, start_hi = sched["start_lo"], sched["start_hi"]

    nc = bacc.Bacc("TRN2", target_bir_lowering=False, debug=False,
                   num_devices=NCORES, num_swdge_queues=NQ)

    def inp(name, shape, dt):
        return nc.declare_dram_parameter(name, list(shape), dt, isOutput=False)

    gidx_lo = inp("gidx_lo", [P, et_lo * 8], i16)
    gidx_hi = inp("gidx_hi", [P, et_hi * 8], i16)
    fp8 = mybir.dt.float8e4
    ohm_lo = inp("ohm_lo", [P, et_lo * P], fp8)
    ohm_hi = inp("ohm_hi", [P, et_hi * P], fp8)
    cnt_in = inp("cnt", [16, NPAD], f32)
    acnt_in = inp("acnt", [16, NPAD], bf16)
    prm_in = inp("prmT", [P, 8 * L], f32)      # cols: l*8 + (b1a,b1b,b2,gamma,beta,0,0,0)
    w1_in = inp("w1", [L, D, 2 * D], f32)
    w2_in = inp("w2", [L, 2 * D, D], f32)
    t9_in = inp("t9", [16, D], bf16)
    t15_in = inp("t15", [L, 16, D], f32)
    idf_in = inp("identf", [P, P], f32)
    idb_in = inp("identb", [P, P], bf16)
    out_ext = nc.declare_dram_parameter("out", [NPC, D], f32, isOutput=True)

    NROWS = NCORES * NPAD
    h_loc = nc.dram_tensor("h_loc", [NROWS, D], bf16, addr_space="Shared")
    h_shard = nc.dram_tensor("h_shard", [NPAD, D], bf16)
    st_loc = nc.dram_tensor("st_loc", [P, 2], f32)
    st_glob = nc.dram_tensor("st_glob", [P, 2], f32, addr_space="Shared")
    RG = [list(range(NCORES))]

    nfull = NPC // P
    rem = NPC - nfull * P
    inv_n = 1.0 / float(N)

    with TileContext(nc) as tc:
        with tc.tile_pool(name="cst", bufs=1) as cp, \
             tc.tile_pool(name="big", bufs=1) as bp, \
             tc.tile_pool(name="nrm", bufs=1) as np_pool, \
             tc.tile_pool(name="wgt", bufs=2) as wp, \
             tc.tile_pool(name="msg", bufs=8) as mp, \
             tc.tile_pool(name="ohp", bufs=4) as op_, \
             tc.tile_pool(name="act", bufs=4) as ap_, \
             tc.tile_pool(name="sml", bufs=1) as sp:

            # ---- persistent constants / inputs in SBUF ----
            def load(pool, shape, dt, src, tag):
                t = pool.tile(list(shape), dt, tag=tag)
                nc.sync.dma_start(out=t[:], in_=src)
                return t

            gi_lo = load(cp, [P, et_lo * 8], i16, gidx_lo[:, :], "gi_lo")
            gi_hi = load(cp, [P, et_hi * 8], i16, gidx_hi[:, :], "gi_hi")

            cnt_sb = load(cp, [16, NPAD], f32, cnt_in[:, :], "cnt")
            acnt_sb = load(cp, [16, NPAD], bf16, acnt_in[:, :], "acnt")
            t9_sb = load(cp, [16, D], bf16, t9_in[:, :], "t9")
            prm_sb = load(cp, [P, 8 * L], f32, prm_in[:, :], "prm")
            idf_sb = load(cp, [P, P], f32, idf_in[:, :], "idf")
            idb_sb = load(cp, [P, P], bf16, idb_in[:, :], "idb")

            # ---- big working buffers ----
            outT = bp.tile([P, NPAD], f32)          # post-MLP, feature-major
            h_nm = bp.tile([P, NT * D], bf16)       # node-major h staging
            hTb = np_pool.tile([P, NPAD], bf16)     # feature-major h (layer input)

            # small stat tiles
            sumc_sb = sp.tile([P, NT], f32)
            sqc_sb = sp.tile([P, NT], f32)
            stats_sb = sp.tile([P, 2], f32)
            gstats_sb = sp.tile([P, 2], f32)
            mean_c = sp.tile([P, 1], f32)
            ex2_c = sp.tile([P, 1], f32)
            msq_c = sp.tile([P, 1], f32)
            var_c = sp.tile([P, 1], f32)
            sd_c = sp.tile([P, 1], f32)
            rstd_c = sp.tile([P, 1], f32)
            k_c = sp.tile([P, 1], f32)
            tmp_c = sp.tile([P, 1], f32)
            c_c = sp.tile([P, 1], f32)

            def store_shard(src_sb, dst_dram):
                """node-major SBUF [128, NT*D] -> DRAM [NPC, D] (strided)."""
                if nfull:
                    nc.sync.dma_start(
                        out=dst_dram[0:nfull * P, :].rearrange("(t p) d -> p t d", p=P),
                        in_=src_sb[:, 0:nfull * D].rearrange("p (t d) -> p t d", d=D))
                if rem:
                    nc.sync.dma_start(
                        out=dst_dram[nfull * P:NPC, :],
                        in_=src_sb[0:rem, nfull * D:(nfull + 1) * D])

            def store_shard_pm(src_sb):
                """SBUF [128, NT*D] -> h_shard [NPAD, D] partition-major, contiguous."""
                nc.sync.dma_start(
                    out=h_shard.ap().rearrange("(p t) d -> p t d", t=NT),
                    in_=src_sb[:].rearrange("p (t d) -> p t d", d=D))

            # ================= embedding phase =================
            # h0 feature-major = t9.T @ acnt (per destination tile), then
            # transpose to node-major for the gather table.
            with tc.tile_pool(name="pse", bufs=2, space="PSUM") as pe_, \
                 tc.tile_pool(name="pst0", bufs=2, space="PSUM") as pt0:
                for nt in range(NT):
                    ps = pe_.tile([P, P], f32, tag="pse")
                    nc.tensor.matmul(out=ps[:], lhsT=t9_sb[:],
                                     rhs=acnt_sb[:, nt * P:(nt + 1) * P],
                                     start=True, stop=True)
                    nc.vector.tensor_scalar_add(
                        hTb[:, nt * P:(nt + 1) * P], ps[:], 0.0)
                    pst = pt0.tile([P, P], bf16, tag="pst0")
                    nc.tensor.transpose(
                        out=pst[:], in_=hTb[:, nt * P:(nt + 1) * P],
                        identity=idb_sb[:])
                    nc.scalar.activation(h_nm[:, nt * D:(nt + 1) * D],
                                         pst[:], ACT.Copy)
            store_shard_pm(h_nm)
            nc.gpsimd.collective_compute(
                "AllGather", mybir.AluOpType.bypass, replica_groups=RG,
                ins=[h_shard.ap().opt()], outs=[h_loc.ap().opt()])

            # ================= layers =================
            for l in range(L):
                w1_sb = load(wp, [D, 2 * D], f32, w1_in[l, :, :], tag="w1")
                w2a_sb = load(wp, [D, D], f32, w2_in[l, 0:D, :], tag="w2a")
                w2b_sb = load(wp, [D, D], f32, w2_in[l, D:2 * D, :], tag="w2b")
                t15_sb = load(wp, [16, D], f32, t15_in[l, :, :], tag="t15")

                b1a = prm_sb[:, l * 8 + 0:l * 8 + 1]
                b1b = prm_sb[:, l * 8 + 1:l * 8 + 2]
                b2 = prm_sb[:, l * 8 + 2:l * 8 + 3]
                gam = prm_sb[:, l * 8 + 3:l * 8 + 4]
                bet = prm_sb[:, l * 8 + 4:l * 8 + 5]

                # lazy chunked gathers + one-hot loads per stream
                chunks = {"lo": {}, "hi": {}}
                stream_cfg = {
                    "lo": (gi_lo, ohm_lo, et_lo, 0),
                    "hi": (gi_hi, ohm_hi, et_hi, VHALF),
                }

                def msg_slice(s, pos):
                    gi, ohm_d, et_s, row0 = stream_cfg[s]
                    ci = pos // CH_TILES
                    if ci not in chunks[s]:
                        ntile = min(CH_TILES, et_s - ci * CH_TILES)
                        nidx = ntile * P
                        g = mp.tile([P, CH_TILES * D], bf16, tag="msg")
                        nc.gpsimd.dma_gather(
                            out_ap=g[:, 0:ntile * D].rearrange("p (t e) -> p t e", e=D),
                            in_ap=h_loc[row0:NROWS, :] if row0 else h_loc[0:VHALF, :],
                            idxs_ap=gi[:, ci * CH_TILES * 8: ci * CH_TILES * 8 + nidx // 16],
                            num_idxs=nidx, num_idxs_reg=nidx, elem_size=D,
                            single_packet=False, queue_num=0)
                        ohc = op_.tile([P, CH_TILES * P], fp8, tag="ohc")
                        t0 = ci * CH_TILES
                        nc.scalar.dma_start(
                            out=ohc[:, 0:ntile * P],
                            in_=ohm_d[:, t0 * P:(t0 + ntile) * P])
                        chunks[s][ci] = (g, ohc)
                    g, ohc = chunks[s][ci]
                    j = pos % CH_TILES
                    return g[:, j * D:(j + 1) * D], ohc[:, j * P:(j + 1) * P]

                with tc.tile_pool(name="psa", bufs=2, space="PSUM") as pa, \
                     tc.tile_pool(name="psh", bufs=2, space="PSUM") as ph, \
                     tc.tile_pool(name="pso", bufs=2, space="PSUM") as po:
                    for nt in range(NT):
                        psa = pa.tile([P, P], f32, tag="psa")
                        first = True
                        for s, karr, starts in (
                            ("lo", k_lo, start_lo),
                            ("hi", k_hi, start_hi),
                        ):
                            for jj in range(karr[nt]):
                                pos = starts[nt] + jj
                                m, oh = msg_slice(s, pos)
                                nc.tensor.matmul(out=psa[:], lhsT=m, rhs=oh,
                                                 start=first, stop=False)
                                first = False
                        nc.tensor.matmul(out=psa[:], lhsT=t15_sb[:],
                                         rhs=cnt_sb[:, nt * P:(nt + 1) * P],
                                         start=first, stop=True)

                        # aggr = psa + h_v (self-loop message), in bf16
                        aggr_b = ap_.tile([P, P], f32, tag="aggr")
                        nc.vector.tensor_tensor(
                            out=aggr_b[:], in0=psa[:],
                            in1=hTb[:, nt * P:(nt + 1) * P], op=ALU.add)

                        psh1 = ph.tile([P, P], f32, tag="psh")
                        nc.tensor.matmul(out=psh1[:], lhsT=w1_sb[:, 0:D],
                                         rhs=aggr_b[:], start=True, stop=True)
                        hidA = ap_.tile([P, P], f32, tag="hidA")
                        nc.scalar.activation(hidA[:], psh1[:], ACT.Relu, bias=b1a)
                        psh2 = ph.tile([P, P], f32, tag="psh")
                        nc.tensor.matmul(out=psh2[:], lhsT=w1_sb[:, D:2 * D],
                                         rhs=aggr_b[:], start=True, stop=True)
                        hidB = ap_.tile([P, P], f32, tag="hidB")
                        nc.scalar.activation(hidB[:], psh2[:], ACT.Relu, bias=b1b)

                        pso1 = po.tile([P, P], f32, tag="pso")
                        nc.tensor.matmul(out=pso1[:], lhsT=w2a_sb[:], rhs=hidA[:],
                                         start=True, stop=False)
                        nc.tensor.matmul(out=pso1[:], lhsT=w2b_sb[:], rhs=hidB[:],
                                         start=False, stop=True)
                        nc.vector.tensor_scalar_add(
                            outT[:, nt * P:(nt + 1) * P], pso1[:], b2)

                        # per-tile BN stat partials (valid node columns only)
                        c0 = nt * P
                        c1 = min((nt + 1) * P, NPC)
                        if c1 > c0:
                            nc.vector.tensor_reduce(
                                out=sumc_sb[:, nt:nt + 1],
                                in_=outT[:, c0:c1],
                                axis=mybir.AxisListType.X, op=ALU.add)
                            sq_scr = ap_.tile([P, P], f32, tag="sqscr")
                            nc.scalar.activation(
                                sq_scr[:, 0:c1 - c0], outT[:, c0:c1], ACT.Square,
                                accum_out=sqc_sb[:, nt:nt + 1])

                # ---- batch-norm statistics: combine tile partials ----
                nc.vector.tensor_reduce(
                    out=stats_sb[:, 0:1], in_=sumc_sb[:],
                    axis=mybir.AxisListType.X, op=ALU.add)
                nc.vector.tensor_reduce(
                    out=stats_sb[:, 1:2], in_=sqc_sb[:],
                    axis=mybir.AxisListType.X, op=ALU.add)
                nc.sync.dma_start(out=st_loc[:, :], in_=stats_sb[:])
                nc.gpsimd.collective_compute(
                    "AllReduce", ALU.add, replica_groups=RG,
                    ins=[st_loc.ap().opt()], outs=[st_glob.ap().opt()])
                nc.sync.dma_start(out=gstats_sb[:], in_=st_glob[:, :])

                nc.vector.tensor_scalar_mul(mean_c[:], gstats_sb[:, 0:1], inv_n)
                nc.vector.tensor_scalar_mul(ex2_c[:], gstats_sb[:, 1:2], inv_n)
                nc.scalar.activation(msq_c[:], mean_c[:], ACT.Square)
                nc.vector.tensor_tensor(var_c[:], ex2_c[:], msq_c[:], op=ALU.subtract)
                nc.vector.tensor_scalar_add(var_c[:], var_c[:], BN_EPS)
                nc.scalar.activation(sd_c[:], var_c[:], ACT.Sqrt)
                nc.vector.reciprocal(rstd_c[:], sd_c[:])
                nc.vector.tensor_tensor(k_c[:], gam, rstd_c[:], op=ALU.mult)
                nc.vector.tensor_tensor(tmp_c[:], mean_c[:], k_c[:], op=ALU.mult)
                nc.vector.tensor_tensor(c_c[:], bet, tmp_c[:], op=ALU.subtract)

                with tc.tile_pool(name="pst", bufs=2, space="PSUM") as pt:
                    if l < L - 1:
                        # h = relu(out*k + c) in bf16, transpose to node-major
                        nc.scalar.activation(hTb[:], outT[:], ACT.Relu,
                                             bias=c_c[:, 0:1], scale=k_c[:, 0:1])
                        for nt in range(NT):
                            pst = pt.tile([P, P], bf16, tag="pst")
                            nc.tensor.transpose(
                                out=pst[:], in_=hTb[:, nt * P:(nt + 1) * P],
                                identity=idb_sb[:])
                            nc.scalar.activation(h_nm[:, nt * D:(nt + 1) * D],
                                                 pst[:], ACT.Copy)
                        store_shard_pm(h_nm)
                        nc.gpsimd.collective_compute(
                            "AllGather", mybir.AluOpType.bypass, replica_groups=RG,
                            ins=[h_shard.ap().opt()], outs=[h_loc.ap().opt()])
                    else:
                        # final: out*k + c in place, transpose, store per tile
                        nc.vector.tensor_scalar(
                            out=outT[:], in0=outT[:],
                            scalar1=k_c[:, 0:1], scalar2=c_c[:, 0:1],
                            op0=ALU.mult, op1=ALU.add)
                        for nt in range(NT):
                            pst = pt.tile([P, P], f32, tag="pstf")
                            nc.tensor.transpose(
                                out=pst[:], in_=outT[:, nt * P:(nt + 1) * P],
                                identity=idf_sb[:])
                            o_t = ap_.tile([P, P], f32, tag="otile")
                            nc.scalar.activation(o_t[:], pst[:], ACT.Copy)
                            r0 = nt * P
                            r1 = min((nt + 1) * P, NPC)
                            if r1 > r0:
                                nc.sync.dma_start(
                                    out=out_ext[r0:r1, :],
                                    in_=o_t[0:r1 - r0, :])

    # Align each gather's SWDGE queue with the DMASW semaphore lane Tile
    # assigned it (lane k <-> queue k % NQ), so no semaphore is shared by
    # two queues (completion order within a lane must match issue order).
    from concourse.tile_scheduler import PROC_NAME_TO_IDX
    dmasw0 = PROC_NAME_TO_IDX["DMASW0"]
    for inst in nc.inst_map.values():
        if isinstance(inst, mybir.InstDMAGatherAnt):
            proc = inst.bass_scheduled_proc
            assert proc is not None and dmasw0 <= proc < dmasw0 + 8, (
                f"gather {inst.name} not on a DMASW lane: {proc}")
            inst.queue_num = (proc - dmasw0) % NQ

    nc.compile()
    return nc


_CACHE = {}


def _make_in_maps(arr, atom_emb0, atom_emb1, edge_emb0, edge_emb1,
                  W1, b1, W2, b2, gamma, beta):
    import ml_dtypes
    # ---- parameter tables (host float prep limited to tiny tables) ----
    ae0 = np.asarray(atom_emb0, np.float32)
    ae1 = np.asarray(atom_emb1, np.float32)
    ee0 = np.asarray(edge_emb0, np.float32)
    ee1 = np.asarray(edge_emb1, np.float32)
    t9 = np.zeros((16, D), np.float32)
    t9[:9] = (ae0[:3, None, :] + ae1[None, :3, :]).reshape(9, D)
    t15 = np.zeros((L, 16, D), np.float32)
    for l in range(L):
        t15[l, :15] = (ee0[l][:, None, :] + ee1[l][None, :, :]).reshape(15, D)

    W1 = np.asarray(W1, np.float32)
    W2 = np.asarray(W2, np.float32)
    b1 = np.asarray(b1, np.float32)
    b2 = np.asarray(b2, np.float32)
    gamma = np.asarray(gamma, np.float32)
    beta = np.asarray(beta, np.float32)
    prmT = np.zeros((P, 8 * L), np.float32)
    for l in range(L):
        prmT[:, l * 8 + 0] = b1[l, 0:D]
        prmT[:, l * 8 + 1] = b1[l, D:2 * D]
        prmT[:, l * 8 + 2] = b2[l]
        prmT[:, l * 8 + 3] = gamma[l]
        prmT[:, l * 8 + 4] = beta[l]

    ident = np.eye(P, dtype=np.float32)
    bf = ml_dtypes.bfloat16

    in_maps = []
    for c in range(NCORES):
        in_maps.append({
            "gidx_lo": _wrap_idx_cols(arr["gidx_lo"][c]),
            "gidx_hi": _wrap_idx_cols(arr["gidx_hi"][c]),
            "ohm_lo": arr["ohm_lo"][c].astype(ml_dtypes.float8_e4m3),
            "ohm_hi": arr["ohm_hi"][c].astype(ml_dtypes.float8_e4m3),
            "cnt": arr["cnt"][c],
            "acnt": arr["acnt"][c].astype(bf),
            "prmT": prmT,
            "w1": W1,
            "w2": W2,
            "t9": t9.astype(bf),
            "t15": t15,
            "identf": ident,
            "identb": ident.astype(bf),
        })
    return in_maps


def kernel(x, edge_index, edge_attr, atom_emb0, atom_emb1,
           edge_emb0, edge_emb1, W1, b1, W2, b2, gamma, beta):
    from concourse.bass_utils import run_bass_kernel_spmd

    sched, arr = _preprocess(x, edge_index, edge_attr)
    key = (tuple(sched["k_lo"]), tuple(sched["k_hi"]))
    if key not in _CACHE:
        _CACHE[key] = _build(sched)
    nc = _CACHE[key]

    in_maps = _make_in_maps(arr, atom_emb0, atom_emb1, edge_emb0, edge_emb1,
                            W1, b1, W2, b2, gamma, beta)
    res = run_bass_kernel_spmd(nc, in_maps, core_ids=list(range(NCORES)))
    out = np.concatenate([res.results[c]["out"] for c in range(NCORES)], axis=0)
    return out.astype(np.float32)


# revision 7
# speedup vs baseline: 1.1185x; 1.0314x over previous
"""AtomGIN (3-layer GIN message passing) on 8 Trainium2 NeuronCores.

Strategy (graph/data parallel, dst-partitioned):
  - Nodes are split evenly across 8 cores (core c owns rows [c*6250, (c+1)*6250)).
  - Each core processes the edges whose DESTINATION is local. Message
    aggregation (segment_sum over incoming edges) is computed as a sequence of
    PE matmuls: for each window of 128 destination nodes, PSUM accumulates
    msg_tile.T @ onehot(dst_offset) over that window's edge tiles.
  - h (node features) is replicated in every core's HBM via AllGather after
    each layer; per-edge source rows are fetched with the GPSIMD dma_gather
    instruction (int16 indices; h is addressed as two halves so indices fit
    in 15 bits). Gathers rotate over the 4 SWDGE queues so descriptor
    generation runs on all four Q7 core pairs concurrently.
  - The destination one-hot matrices are STATIC (edge schedule only), so they
    are precomputed on the host, shipped to HBM, and streamed into SBUF with
    cheap contiguous HWDGE DMAs (no on-device is_equal builds).
  - Self-loop messages are NOT gathered: the feature-major copy of h (hTb,
    kept in SBUF for the transpose path anyway) is added directly into the
    aggregate during the PSUM->SBUF copy. The self-loop edge-embedding term
    stays in the count matmul (code 12 counts).
  - The edge-embedding term is aggregated analytically: for each destination
    node, sum_e eemb[code_e] = table15.T @ counts[:, dst], folded into the
    same PSUM accumulation. The initial atom embedding is likewise a one-hot
    matmul t9.T @ acnt.
  - The per-layer MLP runs in feature-major (transposed) layout so W1/W2 act
    as stationary matmul operands with no activation transposes (W1 stage in
    bf16, W2 stage in f32); BatchNorm statistics are accumulated per
    destination tile (overlapped with the aggregation phase) and combined
    across cores with a tiny AllReduce.
"""

import numpy as np

N = 50000
E = 500000
D = 128
L = 3
BN_EPS = 1e-5
P = 128
NCORES = 8
NPC = N // NCORES            # nodes per core
NT = (NPC + P - 1) // P      # node tiles per core
NPAD = NT * P
VHALF = 32768                # h table split (int16 index headroom)
CH_TILES = 24                # edge tiles per dma_gather call (3072 indices)
NQ = 4                       # SWDGE queues to round-robin


def _wrap_idx_cols(idx2d):
    """[rows] int -> dma_gather wrapped layout [128, rows//16] int16.

    Index i lives at (partition i%16, col i//16), replicated 8x down the
    partition axis (one copy per Q7 core).
    """
    n = idx2d.shape[0]
    w = idx2d.reshape(n // 16, 16).T.astype(np.int16)
    return np.tile(w, (8, 1))


def _preprocess(x, edge_index, edge_attr):
    """Host-side integer preprocessing. Returns (schedule, per-core arrays)."""
    x = np.asarray(x)
    ei = np.asarray(edge_index)
    ea = np.asarray(edge_attr)

    code_a = (x[:, 0] * 3 + x[:, 1]).astype(np.int64)          # [N] in 0..8
    src = ei[0].astype(np.int64)
    dst = ei[1].astype(np.int64)
    ecode = (ea[:, 0] * 3 + ea[:, 1]).astype(np.int64)

    core = dst // NPC
    dst_local = dst - core * NPC
    nt_of_edge = dst_local // P
    # permuted h row for a source node: shards are stored partition-major
    # ([p, t, d] flat), so node (c, i) lands at row c*NPAD + (i%P)*NT + i//P
    s_core = src // NPC
    s_loc = src - s_core * NPC
    grow = s_core * NPAD + (s_loc % P) * NT + s_loc // P
    is_hi = grow >= VHALF

    # --- per (core, node-tile) lo/hi counts -> shared edge-tile schedule ---
    cnt_lo = np.zeros((NCORES, NT), np.int64)
    cnt_hi = np.zeros((NCORES, NT), np.int64)
    np.add.at(cnt_lo, (core[~is_hi], nt_of_edge[~is_hi]), 1)
    np.add.at(cnt_hi, (core[is_hi], nt_of_edge[is_hi]), 1)
    k_lo = np.maximum(1, ((cnt_lo + P - 1) // P).max(axis=0))   # [NT]
    k_hi = np.maximum(1, ((cnt_hi + P - 1) // P).max(axis=0))
    et_lo = int(k_lo.sum())
    et_hi = int(k_hi.sum())
    start_lo = np.zeros(NT + 1, np.int64)
    start_lo[1:] = np.cumsum(k_lo)
    start_hi = np.zeros(NT + 1, np.int64)
    start_hi[1:] = np.cumsum(k_hi)

    gidx_lo = np.zeros((NCORES, et_lo * P), np.int64)
    gidx_hi = np.zeros((NCORES, et_hi * P), np.int64)
    # one-hot tiles [cores, P, et*P]: slot s of tile j maps to dst column
    ohm_lo = np.zeros((NCORES, P, et_lo * P), np.float32)
    ohm_hi = np.zeros((NCORES, P, et_hi * P), np.float32)

    for c in range(NCORES):
        m = core == c
        s_c, dl_c, hi_c = grow[m], dst_local[m], is_hi[m]
        for hi_flag, gidx, ohm, starts in (
            (False, gidx_lo, ohm_lo, start_lo),
            (True, gidx_hi, ohm_hi, start_hi),
        ):
            sel = hi_c == hi_flag
            s_s = s_c[sel] - (VHALF if hi_flag else 0)
            dl_s = dl_c[sel]
            order = np.argsort(dl_s, kind="stable")
            s_s, dl_s = s_s[order], dl_s[order]
            nt_s = dl_s // P
            seg_first = np.searchsorted(nt_s, np.arange(NT))
            rank = np.arange(len(dl_s)) - seg_first[nt_s]
            tile_of = starts[nt_s] + rank // P
            slot = tile_of * P + rank % P
            gidx[c, slot] = s_s
            ohm[c, rank % P, tile_of * P + (dl_s - nt_s * P)] = 1.0

    # edge-code count matrix [cores, 16, NPAD] (codes 0..14; row 15 zero-pad)
    # self-loop edges (code 4*3+0=12, one per node) contribute only here.
    cnt = np.zeros((NCORES, 16, NPAD), np.float32)
    np.add.at(cnt, (core, ecode, dst_local), 1.0)
    loop_l = np.arange(N, dtype=np.int64) % NPC
    cnt[np.arange(N) // NPC, 12, loop_l] += 1.0

    # atom-code one-hot [cores, 16, NPAD]
    acnt = np.zeros((NCORES, 16, NPAD), np.float32)
    acnt[np.arange(N) // NPC, code_a, loop_l] = 1.0

    sched = dict(k_lo=k_lo.tolist(), k_hi=k_hi.tolist(),
                 et_lo=et_lo, et_hi=et_hi,
                 start_lo=start_lo.tolist(), start_hi=start_hi.tolist())
    arrays = dict(gidx_lo=gidx_lo, gidx_hi=gidx_hi,
                  ohm_lo=ohm_lo, ohm_hi=ohm_hi,
                  cnt=cnt, acnt=acnt)
    return sched, arrays


def _build(sched):
    """Build the SPMD Bacc graph (one graph, run on all 8 cores)."""
    import concourse.bacc as bacc
    import concourse.mybir as mybir
    from concourse.tile import TileContext

    f32 = mybir.dt.float32
    bf16 = mybir.dt.bfloat16
    i16 = mybir.dt.int16
    ACT = mybir.ActivationFunctionType
    ALU = mybir.AluOpType

    k_lo, k_hi = sched["k_lo"], sched["k_hi"]
    et_lo, et_hi = sched["et_lo"], sched["et_hi"]
    start_lo, start_hi = sched["start_lo"], sched["start_hi"]

    nc = bacc.Bacc("TRN2", target_bir_lowering=False, debug=False,
                   num_devices=NCORES, num_swdge_queues=NQ)

    def inp(name, shape, dt):
        return nc.declare_dram_parameter(name, list(shape), dt, isOutput=False)

    gidx_lo = inp("gidx_lo", [P, et_lo * 8], i16)
    gidx_hi = inp("gidx_hi", [P, et_hi * 8], i16)
    fp8 = mybir.dt.float8e4
    ohm_lo = inp("ohm_lo", [P, et_lo * P], fp8)
    ohm_hi = inp("ohm_hi", [P, et_hi * P], fp8)
    cnt_in = inp("cnt", [16, NPAD], f32)
    acnt_in = inp("acnt", [16, NPAD], bf16)
    prm_in = inp("prmT", [P, 8 * L], f32)      # cols: l*8 + (b1a,b1b,b2,gamma,beta,0,0,0)
    w1_in = inp("w1", [L, D, 2 * D], f32)
    w2_in = inp("w2", [L, 2 * D, D], f32)
    t9_in = inp("t9", [16, D], bf16)
    t15_in = inp("t15", [L, 16, D], f32)
    idf_in = inp("identf", [P, P], f32)
    idb_in = inp("identb", [P, P], bf16)
    out_ext = nc.declare_dram_parameter("out", [NPC, D], f32, isOutput=True)

    NROWS = NCORES * NPAD
    h_loc = nc.dram_tensor("h_loc", [NROWS, D], bf16, addr_space="Shared")
    h_shard = nc.dram_tensor("h_shard", [NPAD, D], bf16)
    st_loc = nc.dram_tensor("st_loc", [P, 2], f32)
    st_glob = nc.dram_tensor("st_glob", [P, 2], f32, addr_space="Shared")
    RG = [list(range(NCORES))]

    nfull = NPC // P
    rem = NPC - nfull * P
    inv_n = 1.0 / float(N)

    with TileContext(nc) as tc:
        with tc.tile_pool(name="cst", bufs=1) as cp, \
             tc.tile_pool(name="big", bufs=1) as bp, \
             tc.tile_pool(name="nrm", bufs=1) as np_pool, \
             tc.tile_pool(name="wgt", bufs=2) as wp, \
             tc.tile_pool(name="msg", bufs=8) as mp, \
             tc.tile_pool(name="ohp", bufs=4) as op_, \
             tc.tile_pool(name="act", bufs=4) as ap_, \
             tc.tile_pool(name="sml", bufs=1) as sp:

            # ---- persistent constants / inputs in SBUF ----
            def load(pool, shape, dt, src, tag):
                t = pool.tile(list(shape), dt, tag=tag)
                nc.sync.dma_start(out=t[:], in_=src)
                return t

            gi_lo = load(cp, [P, et_lo * 8], i16, gidx_lo[:, :], "gi_lo")
            gi_hi = load(cp, [P, et_hi * 8], i16, gidx_hi[:, :], "gi_hi")

            cnt_sb = load(cp, [16, NPAD], f32, cnt_in[:, :], "cnt")
            acnt_sb = load(cp, [16, NPAD], bf16, acnt_in[:, :], "acnt")
            t9_sb = load(cp, [16, D], bf16, t9_in[:, :], "t9")
            prm_sb = load(cp, [P, 8 * L], f32, prm_in[:, :], "prm")
            idf_sb = load(cp, [P, P], f32, idf_in[:, :], "idf")
            idb_sb = load(cp, [P, P], bf16, idb_in[:, :], "idb")

            # ---- big working buffers ----
            outT = bp.tile([P, NPAD], f32)          # post-MLP, feature-major
            h_nm = bp.tile([P, NT * D], bf16)       # node-major h staging
            hTb = np_pool.tile([P, NPAD], bf16)     # feature-major h (layer input)

            # small stat tiles
            sumc_sb = sp.tile([P, NT], f32)
            sqc_sb = sp.tile([P, NT], f32)
            stats_sb = sp.tile([P, 2], f32)
            gstats_sb = sp.tile([P, 2], f32)
            mean_c = sp.tile([P, 1], f32)
            ex2_c = sp.tile([P, 1], f32)
            msq_c = sp.tile([P, 1], f32)
            var_c = sp.tile([P, 1], f32)
            sd_c = sp.tile([P, 1], f32)
            rstd_c = sp.tile([P, 1], f32)
            k_c = sp.tile([P, 1], f32)
            tmp_c = sp.tile([P, 1], f32)
            c_c = sp.tile([P, 1], f32)

            def store_shard(src_sb, dst_dram):
                """node-major SBUF [128, NT*D] -> DRAM [NPC, D] (strided)."""
                if nfull:
                    nc.sync.dma_start(
                        out=dst_dram[0:nfull * P, :].rearrange("(t p) d -> p t d", p=P),
                        in_=src_sb[:, 0:nfull * D].rearrange("p (t d) -> p t d", d=D))
                if rem:
                    nc.sync.dma_start(
                        out=dst_dram[nfull * P:NPC, :],
                        in_=src_sb[0:rem, nfull * D:(nfull + 1) * D])

            def store_shard_pm(src_sb):
                """SBUF [128, NT*D] -> h_shard [NPAD, D] partition-major, contiguous."""
                nc.sync.dma_start(
                    out=h_shard.ap().rearrange("(p t) d -> p t d", t=NT),
                    in_=src_sb[:].rearrange("p (t d) -> p t d", d=D))

            # ================= embedding phase =================
            # h0 feature-major = t9.T @ acnt (per destination tile), then
            # transpose to node-major for the gather table.
            with tc.tile_pool(name="pse", bufs=2, space="PSUM") as pe_, \
                 tc.tile_pool(name="pst0", bufs=2, space="PSUM") as pt0:
                for nt in range(NT):
                    ps = pe_.tile([P, P], f32, tag="pse")
                    nc.tensor.matmul(out=ps[:], lhsT=t9_sb[:],
                                     rhs=acnt_sb[:, nt * P:(nt + 1) * P],
                                     start=True, stop=True)
                    nc.vector.tensor_scalar_add(
                        hTb[:, nt * P:(nt + 1) * P], ps[:], 0.0)
                    pst = pt0.tile([P, P], bf16, tag="pst0")
                    nc.tensor.transpose(
                        out=pst[:], in_=hTb[:, nt * P:(nt + 1) * P],
                        identity=idb_sb[:])
                    nc.scalar.activation(h_nm[:, nt * D:(nt + 1) * D],
                                         pst[:], ACT.Copy)
            store_shard_pm(h_nm)
            nc.gpsimd.collective_compute(
                "AllGather", mybir.AluOpType.bypass, replica_groups=RG,
                ins=[h_shard.ap().opt()], outs=[h_loc.ap().opt()])

            # ================= layers =================
            for l in range(L):
                w1_sb = load(wp, [D, 2 * D], f32, w1_in[l, :, :], tag="w1")
                w2a_sb = load(wp, [D, D], f32, w2_in[l, 0:D, :], tag="w2a")
                w2b_sb = load(wp, [D, D], f32, w2_in[l, D:2 * D, :], tag="w2b")
                t15_sb = load(wp, [16, D], f32, t15_in[l, :, :], tag="t15")

                b1a = prm_sb[:, l * 8 + 0:l * 8 + 1]
                b1b = prm_sb[:, l * 8 + 1:l * 8 + 2]
                b2 = prm_sb[:, l * 8 + 2:l * 8 + 3]
                gam = prm_sb[:, l * 8 + 3:l * 8 + 4]
                bet = prm_sb[:, l * 8 + 4:l * 8 + 5]

                # lazy chunked gathers + one-hot loads per stream; the
                # first chunks are small so the first PE chains unblock
                # quickly after the AllGather.
                def chunk_plan(et_s):
                    plan, t0 = [], 0
                    for sz in (8, 8, 16):
                        if t0 >= et_s:
                            break
                        sz = min(sz, et_s - t0)
                        plan.append((t0, sz))
                        t0 += sz
                    while t0 < et_s:
                        sz = min(CH_TILES, et_s - t0)
                        plan.append((t0, sz))
                        t0 += sz
                    tile2chunk = []
                    for ci, (c0, sz) in enumerate(plan):
                        tile2chunk += [ci] * sz
                    return plan, tile2chunk

                chunks = {"lo": {}, "hi": {}}
                stream_cfg = {
                    "lo": (gi_lo, ohm_lo, chunk_plan(et_lo), 0),
                    "hi": (gi_hi, ohm_hi, chunk_plan(et_hi), VHALF),
                }

                def msg_slice(s, pos):
                    gi, ohm_d, (plan, t2c), row0 = stream_cfg[s]
                    ci = t2c[pos]
                    if ci not in chunks[s]:
                        t0, ntile = plan[ci]
                        nidx = ntile * P
                        g = mp.tile([P, CH_TILES * D], bf16, tag="msg")
                        nc.gpsimd.dma_gather(
                            out_ap=g[:, 0:ntile * D].rearrange("p (t e) -> p t e", e=D),
                            in_ap=h_loc[row0:NROWS, :] if row0 else h_loc[0:VHALF, :],
                            idxs_ap=gi[:, t0 * 8: t0 * 8 + nidx // 16],
                            num_idxs=nidx, num_idxs_reg=nidx, elem_size=D,
                            single_packet=False, queue_num=0)
                        ohc = op_.tile([P, CH_TILES * P], fp8, tag="ohc")
                        nc.scalar.dma_start(
                            out=ohc[:, 0:ntile * P],
                            in_=ohm_d[:, t0 * P:(t0 + ntile) * P])
                        chunks[s][ci] = (g, ohc, t0)
                    g, ohc, t0 = chunks[s][ci]
                    j = pos - t0
                    return g[:, j * D:(j + 1) * D], ohc[:, j * P:(j + 1) * P]

                with tc.tile_pool(name="psa", bufs=2, space="PSUM") as pa, \
                     tc.tile_pool(name="psh", bufs=2, space="PSUM") as ph, \
                     tc.tile_pool(name="pso", bufs=2, space="PSUM") as po:
                    for nt in range(NT):
                        psa = pa.tile([P, P], f32, tag="psa")
                        first = True
                        for s, karr, starts in (
                            ("lo", k_lo, start_lo),
                            ("hi", k_hi, start_hi),
                        ):
                            for jj in range(karr[nt]):
                                pos = starts[nt] + jj
                                m, oh = msg_slice(s, pos)
                                nc.tensor.matmul(out=psa[:], lhsT=m, rhs=oh,
                                                 start=first, stop=False)
                                first = False
                        nc.tensor.matmul(out=psa[:], lhsT=t15_sb[:],
                                         rhs=cnt_sb[:, nt * P:(nt + 1) * P],
                                         start=first, stop=True)

                        # aggr = psa + h_v (self-loop message), in bf16
                        aggr_b = ap_.tile([P, P], f32, tag="aggr")
                        nc.vector.tensor_tensor(
                            out=aggr_b[:], in0=psa[:],
                            in1=hTb[:, nt * P:(nt + 1) * P], op=ALU.add)

                        psh1 = ph.tile([P, P], f32, tag="psh")
                        nc.tensor.matmul(out=psh1[:], lhsT=w1_sb[:, 0:D],
                                         rhs=aggr_b[:], start=True, stop=True)
                        hidA = ap_.tile([P, P], f32, tag="hidA")
                        nc.scalar.activation(hidA[:], psh1[:], ACT.Relu, bias=b1a)
                        psh2 = ph.tile([P, P], f32, tag="psh")
                        nc.tensor.matmul(out=psh2[:], lhsT=w1_sb[:, D:2 * D],
                                         rhs=aggr_b[:], start=True, stop=True)
                        hidB = ap_.tile([P, P], f32, tag="hidB")
                        nc.scalar.activation(hidB[:], psh2[:], ACT.Relu, bias=b1b)

                        pso1 = po.tile([P, P], f32, tag="pso")
                        nc.tensor.matmul(out=pso1[:], lhsT=w2a_sb[:], rhs=hidA[:],
                                         start=True, stop=False)
                        nc.tensor.matmul(out=pso1[:], lhsT=w2b_sb[:], rhs=hidB[:],
                                         start=False, stop=True)
                        nc.vector.tensor_scalar_add(
                            outT[:, nt * P:(nt + 1) * P], pso1[:], b2)

                        # per-tile BN stat partials (valid node columns only)
                        c0 = nt * P
                        c1 = min((nt + 1) * P, NPC)
                        if c1 > c0:
                            nc.vector.tensor_reduce(
                                out=sumc_sb[:, nt:nt + 1],
                                in_=outT[:, c0:c1],
                                axis=mybir.AxisListType.X, op=ALU.add)
                            sq_scr = ap_.tile([P, P], f32, tag="sqscr")
                            nc.scalar.activation(
                                sq_scr[:, 0:c1 - c0], outT[:, c0:c1], ACT.Square,
                                accum_out=sqc_sb[:, nt:nt + 1])

                # ---- batch-norm statistics: combine tile partials ----
                nc.vector.tensor_reduce(
                    out=stats_sb[:, 0:1], in_=sumc_sb[:],
                    axis=mybir.AxisListType.X, op=ALU.add)
                nc.vector.tensor_reduce(
                    out=stats_sb[:, 1:2], in_=sqc_sb[:],
                    axis=mybir.AxisListType.X, op=ALU.add)
                nc.sync.dma_start(out=st_loc[:, :], in_=stats_sb[:])
                nc.gpsimd.collective_compute(
                    "AllReduce", ALU.add, replica_groups=RG,
                    ins=[st_loc.ap().opt()], outs=[st_glob.ap().opt()])
                nc.sync.dma_start(out=gstats_sb[:], in_=st_glob[:, :])

                nc.vector.tensor_scalar_mul(mean_c[:], gstats_sb[:, 0:1], inv_n)
                nc.vector.tensor_scalar_mul(ex2_c[:], gstats_sb[:, 1:2], inv_n)
                nc.scalar.activation(msq_c[:], mean_c[:], ACT.Square)
                nc.vector.tensor_tensor(var_c[:], ex2_c[:], msq_c[:], op=ALU.subtract)
                nc.vector.tensor_scalar_add(var_c[:], var_c[:], BN_EPS)
                nc.scalar.activation(sd_c[:], var_c[:], ACT.Sqrt)
                nc.vector.reciprocal(rstd_c[:], sd_c[:])
                nc.vector.tensor_tensor(k_c[:], gam, rstd_c[:], op=ALU.mult)
                nc.vector.tensor_tensor(tmp_c[:], mean_c[:], k_c[:], op=ALU.mult)
                nc.vector.tensor_tensor(c_c[:], bet, tmp_c[:], op=ALU.subtract)

                with tc.tile_pool(name="pst", bufs=2, space="PSUM") as pt:
                    if l < L - 1:
                        # h = relu(out*k + c) in bf16, transpose to node-major
                        nc.scalar.activation(hTb[:], outT[:], ACT.Relu,
                                             bias=c_c[:, 0:1], scale=k_c[:, 0:1])
                        for nt in range(NT):
                            pst = pt.tile([P, P], bf16, tag="pst")
                            nc.tensor.transpose(
                                out=pst[:], in_=hTb[:, nt * P:(nt + 1) * P],
                                identity=idb_sb[:])
                            nc.scalar.activation(h_nm[:, nt * D:(nt + 1) * D],
                                                 pst[:], ACT.Copy)
                        store_shard_pm(h_nm)
                        nc.gpsimd.collective_compute(
                            "AllGather", mybir.AluOpType.bypass, replica_groups=RG,
                            ins=[h_shard.ap().opt()], outs=[h_loc.ap().opt()])
                    else:
                        # final: out*k + c in place, transpose, store per tile
                        nc.vector.tensor_scalar(
                            out=outT[:], in0=outT[:],
                            scalar1=k_c[:, 0:1], scalar2=c_c[:, 0:1],
                            op0=ALU.mult, op1=ALU.add)
                        for nt in range(NT):
                            pst = pt.tile([P, P], f32, tag="pstf")
                            nc.tensor.transpose(
                                out=pst[:], in_=outT[:, nt * P:(nt + 1) * P],
                                identity=idf_sb[:])
                            o_t = ap_.tile([P, P], f32, tag="otile")
                            nc.scalar.activation(o_t[:], pst[:], ACT.Copy)
                            r0 = nt * P
                            r1 = min((nt + 1) * P, NPC)
                            if r1 > r0:
                                nc.sync.dma_start(
                                    out=out_ext[r0:r1, :],
                                    in_=o_t[0:r1 - r0, :])

    # Align each gather's SWDGE queue with the DMASW semaphore lane Tile
    # assigned it (lane k <-> queue k % NQ), so no semaphore is shared by
    # two queues (completion order within a lane must match issue order).
    from concourse.tile_scheduler import PROC_NAME_TO_IDX
    dmasw0 = PROC_NAME_TO_IDX["DMASW0"]
    for inst in nc.inst_map.values():
        if isinstance(inst, mybir.InstDMAGatherAnt):
            proc = inst.bass_scheduled_proc
            assert proc is not None and dmasw0 <= proc < dmasw0 + 8, (
                f"gather {inst.name} not on a DMASW lane: {proc}")
            inst.queue_num = (proc - dmasw0) % NQ

    nc.compile()
    return nc


_CACHE = {}


def _make_in_maps(arr, atom_emb0, atom_emb1, edge_emb0, edge_emb1,
                  W1, b1, W2, b2, gamma, beta):
    import ml_dtypes
    # ---- parameter tables (host float prep limited to tiny tables) ----
    ae0 = np.asarray(atom_emb0, np.float32)
    ae1 = np.asarray(atom_emb1, np.float32)
    ee0 = np.asarray(edge_emb0, np.float32)
    ee1 = np.asarray(edge_emb1, np.float32)
    t9 = np.zeros((16, D), np.float32)
    t9[:9] = (ae0[:3, None, :] + ae1[None, :3, :]).reshape(9, D)
    t15 = np.zeros((L, 16, D), np.float32)
    for l in range(L):
        t15[l, :15] = (ee0[l][:, None, :] + ee1[l][None, :, :]).reshape(15, D)

    W1 = np.asarray(W1, np.float32)
    W2 = np.asarray(W2, np.float32)
    b1 = np.asarray(b1, np.float32)
    b2 = np.asarray(b2, np.float32)
    gamma = np.asarray(gamma, np.float32)
    beta = np.asarray(beta, np.float32)
    prmT = np.zeros((P, 8 * L), np.float32)
    for l in range(L):
        prmT[:, l * 8 + 0] = b1[l, 0:D]
        prmT[:, l * 8 + 1] = b1[l, D:2 * D]
        prmT[:, l * 8 + 2] = b2[l]
        prmT[:, l * 8 + 3] = gamma[l]
        prmT[:, l * 8 + 4] = beta[l]

    ident = np.eye(P, dtype=np.float32)
    bf = ml_dtypes.bfloat16

    in_maps = []
    for c in range(NCORES):
        in_maps.append({
            "gidx_lo": _wrap_idx_cols(arr["gidx_lo"][c]),
            "gidx_hi": _wrap_idx_cols(arr["gidx_hi"][c]),
            "ohm_lo": arr["ohm_lo"][c].astype(ml_dtypes.float8_e4m3),
            "ohm_hi": arr["ohm_hi"][c].astype(ml_dtypes.float8_e4m3),
            "cnt": arr["cnt"][c],
            "acnt": arr["acnt"][c].astype(bf),
            "prmT": prmT,
            "w1": W1,
            "w2": W2,
            "t9": t9.astype(bf),
            "t15": t15,
            "identf": ident,
            "identb": ident.astype(bf),
        })
    return in_maps


def kernel(x, edge_index, edge_attr, atom_emb0, atom_emb1,
           edge_emb0, edge_emb1, W1, b1, W2, b2, gamma, beta):
    from concourse.bass_utils import run_bass_kernel_spmd

    sched, arr = _preprocess(x, edge_index, edge_attr)
    key = (tuple(sched["k_lo"]), tuple(sched["k_hi"]))
    if key not in _CACHE:
        _CACHE[key] = _build(sched)
    nc = _CACHE[key]

    in_maps = _make_in_maps(arr, atom_emb0, atom_emb1, edge_emb0, edge_emb1,
                            W1, b1, W2, b2, gamma, beta)
    res = run_bass_kernel_spmd(nc, in_maps, core_ids=list(range(NCORES)))
    out = np.concatenate([res.results[c]["out"] for c in range(NCORES)], axis=0)
    return out.astype(np.float32)


# revision 8
# speedup vs baseline: 1.1408x; 1.0199x over previous
"""AtomGIN (3-layer GIN message passing) on 8 Trainium2 NeuronCores.

Strategy (graph/data parallel, dst-partitioned):
  - Nodes are split evenly across 8 cores (core c owns rows [c*6250, (c+1)*6250)).
  - Each core processes the edges whose DESTINATION is local. Message
    aggregation (segment_sum over incoming edges) is computed as a sequence of
    PE matmuls: for each window of 128 destination nodes, PSUM accumulates
    msg_tile.T @ onehot(dst_offset) over that window's edge tiles.
  - h (node features) is replicated in every core's HBM via AllGather after
    each layer; per-edge source rows are fetched with the GPSIMD dma_gather
    instruction (int16 indices; h is addressed as two halves so indices fit
    in 15 bits). Gathers rotate over the 4 SWDGE queues so descriptor
    generation runs on all four Q7 core pairs concurrently.
  - The destination one-hot matrices are STATIC (edge schedule only), so they
    are precomputed on the host, shipped to HBM, and streamed into SBUF with
    cheap contiguous HWDGE DMAs (no on-device is_equal builds).
  - Self-loop messages are NOT gathered: the feature-major copy of h (hTb,
    kept in SBUF for the transpose path anyway) is added directly into the
    aggregate during the PSUM->SBUF copy. The self-loop edge-embedding term
    stays in the count matmul (code 12 counts).
  - The edge-embedding term is aggregated analytically: for each destination
    node, sum_e eemb[code_e] = table15.T @ counts[:, dst], folded into the
    same PSUM accumulation. The initial atom embedding is likewise a one-hot
    matmul t9.T @ acnt.
  - The per-layer MLP runs in feature-major (transposed) layout so W1/W2 act
    as stationary matmul operands with no activation transposes (W1 stage in
    bf16, W2 stage in f32); BatchNorm statistics are accumulated per
    destination tile (overlapped with the aggregation phase) and combined
    across cores with a tiny AllReduce.
"""

import numpy as np

N = 50000
E = 500000
D = 128
L = 3
BN_EPS = 1e-5
P = 128
NCORES = 8
NPC = N // NCORES            # nodes per core
NT = (NPC + P - 1) // P      # node tiles per core
NPAD = NT * P
VHALF = 32768                # h table split (int16 index headroom)
CH_TILES = 12                # edge tiles per dma_gather call (1536 indices)
NQ = 4                       # SWDGE queues to round-robin


def _wrap_idx_cols(idx2d):
    """[rows] int -> dma_gather wrapped layout [128, rows//16] int16.

    Index i lives at (partition i%16, col i//16), replicated 8x down the
    partition axis (one copy per Q7 core).
    """
    n = idx2d.shape[0]
    w = idx2d.reshape(n // 16, 16).T.astype(np.int16)
    return np.tile(w, (8, 1))


def _preprocess(x, edge_index, edge_attr):
    """Host-side integer preprocessing. Returns (schedule, per-core arrays)."""
    x = np.asarray(x)
    ei = np.asarray(edge_index)
    ea = np.asarray(edge_attr)

    code_a = (x[:, 0] * 3 + x[:, 1]).astype(np.int64)          # [N] in 0..8
    src = ei[0].astype(np.int64)
    dst = ei[1].astype(np.int64)
    ecode = (ea[:, 0] * 3 + ea[:, 1]).astype(np.int64)

    core = dst // NPC
    dst_local = dst - core * NPC
    nt_of_edge = dst_local // P
    # permuted h row for a source node: shards are stored partition-major
    # ([p, t, d] flat), so node (c, i) lands at row c*NPAD + (i%P)*NT + i//P
    s_core = src // NPC
    s_loc = src - s_core * NPC
    grow = s_core * NPAD + (s_loc % P) * NT + s_loc // P
    is_hi = grow >= VHALF

    # --- per (core, node-tile) lo/hi counts -> shared edge-tile schedule ---
    cnt_lo = np.zeros((NCORES, NT), np.int64)
    cnt_hi = np.zeros((NCORES, NT), np.int64)
    np.add.at(cnt_lo, (core[~is_hi], nt_of_edge[~is_hi]), 1)
    np.add.at(cnt_hi, (core[is_hi], nt_of_edge[is_hi]), 1)
    k_lo = np.maximum(1, ((cnt_lo + P - 1) // P).max(axis=0))   # [NT]
    k_hi = np.maximum(1, ((cnt_hi + P - 1) // P).max(axis=0))
    et_lo = int(k_lo.sum())
    et_hi = int(k_hi.sum())
    start_lo = np.zeros(NT + 1, np.int64)
    start_lo[1:] = np.cumsum(k_lo)
    start_hi = np.zeros(NT + 1, np.int64)
    start_hi[1:] = np.cumsum(k_hi)

    gidx_lo = np.zeros((NCORES, et_lo * P), np.int64)
    gidx_hi = np.zeros((NCORES, et_hi * P), np.int64)
    # one-hot tiles [cores, P, et*P]: slot s of tile j maps to dst column
    ohm_lo = np.zeros((NCORES, P, et_lo * P), np.float32)
    ohm_hi = np.zeros((NCORES, P, et_hi * P), np.float32)

    for c in range(NCORES):
        m = core == c
        s_c, dl_c, hi_c = grow[m], dst_local[m], is_hi[m]
        for hi_flag, gidx, ohm, starts in (
            (False, gidx_lo, ohm_lo, start_lo),
            (True, gidx_hi, ohm_hi, start_hi),
        ):
            sel = hi_c == hi_flag
            s_s = s_c[sel] - (VHALF if hi_flag else 0)
            dl_s = dl_c[sel]
            order = np.argsort(dl_s, kind="stable")
            s_s, dl_s = s_s[order], dl_s[order]
            nt_s = dl_s // P
            seg_first = np.searchsorted(nt_s, np.arange(NT))
            rank = np.arange(len(dl_s)) - seg_first[nt_s]
            tile_of = starts[nt_s] + rank // P
            slot = tile_of * P + rank % P
            gidx[c, slot] = s_s
            ohm[c, rank % P, tile_of * P + (dl_s - nt_s * P)] = 1.0

    # edge-code count matrix [cores, 16, NPAD] (codes 0..14; row 15 zero-pad)
    # self-loop edges (code 4*3+0=12, one per node) contribute only here.
    cnt = np.zeros((NCORES, 16, NPAD), np.float32)
    np.add.at(cnt, (core, ecode, dst_local), 1.0)
    loop_l = np.arange(N, dtype=np.int64) % NPC
    cnt[np.arange(N) // NPC, 12, loop_l] += 1.0

    # atom-code one-hot [cores, 16, NPAD]
    acnt = np.zeros((NCORES, 16, NPAD), np.float32)
    acnt[np.arange(N) // NPC, code_a, loop_l] = 1.0

    sched = dict(k_lo=k_lo.tolist(), k_hi=k_hi.tolist(),
                 et_lo=et_lo, et_hi=et_hi,
                 start_lo=start_lo.tolist(), start_hi=start_hi.tolist())
    arrays = dict(gidx_lo=gidx_lo, gidx_hi=gidx_hi,
                  ohm_lo=ohm_lo, ohm_hi=ohm_hi,
                  cnt=cnt, acnt=acnt)
    return sched, arrays


def _build(sched):
    """Build the SPMD Bacc graph (one graph, run on all 8 cores)."""
    import concourse.bacc as bacc
    import concourse.mybir as mybir
    from concourse.tile import TileContext

    f32 = mybir.dt.float32
    bf16 = mybir.dt.bfloat16
    i16 = mybir.dt.int16
    ACT = mybir.ActivationFunctionType
    ALU = mybir.AluOpType

    k_lo, k_hi = sched["k_lo"], sched["k_hi"]
    et_lo, et_hi = sched["et_lo"], sched["et_hi"]
    start_lo, start_hi = sched["start_lo"], sched["start_hi"]

    nc = bacc.Bacc("TRN2", target_bir_lowering=False, debug=False,
                   num_devices=NCORES, num_swdge_queues=NQ)

    def inp(name, shape, dt):
        return nc.declare_dram_parameter(name, list(shape), dt, isOutput=False)

    gidx_lo = inp("gidx_lo", [P, et_lo * 8], i16)
    gidx_hi = inp("gidx_hi", [P, et_hi * 8], i16)
    fp8 = mybir.dt.float8e4
    ohm_lo = inp("ohm_lo", [P, et_lo * P], fp8)
    ohm_hi = inp("ohm_hi", [P, et_hi * P], fp8)
    cnt_in = inp("cnt", [16, NPAD], f32)
    acnt_in = inp("acnt", [16, NPAD], bf16)
    prm_in = inp("prmT", [P, 8 * L], f32)      # cols: l*8 + (b1a,b1b,b2,gamma,beta,0,0,0)
    w1_in = inp("w1", [L, D, 2 * D], f32)
    w2_in = inp("w2", [L, 2 * D, D], f32)
    t9_in = inp("t9", [16, D], bf16)
    t15_in = inp("t15", [L, 16, D], f32)
    idf_in = inp("identf", [P, P], f32)
    idb_in = inp("identb", [P, P], bf16)
    out_ext = nc.declare_dram_parameter("out", [NPC, D], f32, isOutput=True)

    NROWS = NCORES * NPAD
    h_loc = nc.dram_tensor("h_loc", [NROWS, D], bf16, addr_space="Shared")
    h_shard = nc.dram_tensor("h_shard", [NPAD, D], bf16)
    st_loc = nc.dram_tensor("st_loc", [P, 2], f32)
    st_glob = nc.dram_tensor("st_glob", [P, 2], f32, addr_space="Shared")
    RG = [list(range(NCORES))]

    nfull = NPC // P
    rem = NPC - nfull * P
    inv_n = 1.0 / float(N)

    with TileContext(nc) as tc:
        with tc.tile_pool(name="cst", bufs=1) as cp, \
             tc.tile_pool(name="big", bufs=1) as bp, \
             tc.tile_pool(name="nrm", bufs=1) as np_pool, \
             tc.tile_pool(name="wgt", bufs=2) as wp, \
             tc.tile_pool(name="msg", bufs=12) as mp, \
             tc.tile_pool(name="ohp", bufs=8) as op_, \
             tc.tile_pool(name="act", bufs=4) as ap_, \
             tc.tile_pool(name="sml", bufs=1) as sp:

            # ---- persistent constants / inputs in SBUF ----
            def load(pool, shape, dt, src, tag):
                t = pool.tile(list(shape), dt, tag=tag)
                nc.sync.dma_start(out=t[:], in_=src)
                return t

            gi_lo = load(cp, [P, et_lo * 8], i16, gidx_lo[:, :], "gi_lo")
            gi_hi = load(cp, [P, et_hi * 8], i16, gidx_hi[:, :], "gi_hi")

            cnt_sb = load(cp, [16, NPAD], f32, cnt_in[:, :], "cnt")
            acnt_sb = load(cp, [16, NPAD], bf16, acnt_in[:, :], "acnt")
            t9_sb = load(cp, [16, D], bf16, t9_in[:, :], "t9")
            prm_sb = load(cp, [P, 8 * L], f32, prm_in[:, :], "prm")
            idf_sb = load(cp, [P, P], f32, idf_in[:, :], "idf")
            idb_sb = load(cp, [P, P], bf16, idb_in[:, :], "idb")

            # ---- big working buffers ----
            outT = bp.tile([P, NPAD], f32)          # post-MLP, feature-major
            h_nm = bp.tile([P, NT * D], bf16)       # node-major h staging
            hTb = np_pool.tile([P, NPAD], bf16)     # feature-major h (layer input)

            # small stat tiles
            sumc_sb = sp.tile([P, NT], f32)
            sqc_sb = sp.tile([P, NT], f32)
            stats_sb = sp.tile([P, 2], f32)
            gstats_sb = sp.tile([P, 2], f32)
            mean_c = sp.tile([P, 1], f32)
            ex2_c = sp.tile([P, 1], f32)
            msq_c = sp.tile([P, 1], f32)
            var_c = sp.tile([P, 1], f32)
            sd_c = sp.tile([P, 1], f32)
            rstd_c = sp.tile([P, 1], f32)
            k_c = sp.tile([P, 1], f32)
            tmp_c = sp.tile([P, 1], f32)
            c_c = sp.tile([P, 1], f32)

            def store_shard(src_sb, dst_dram):
                """node-major SBUF [128, NT*D] -> DRAM [NPC, D] (strided)."""
                if nfull:
                    nc.sync.dma_start(
                        out=dst_dram[0:nfull * P, :].rearrange("(t p) d -> p t d", p=P),
                        in_=src_sb[:, 0:nfull * D].rearrange("p (t d) -> p t d", d=D))
                if rem:
                    nc.sync.dma_start(
                        out=dst_dram[nfull * P:NPC, :],
                        in_=src_sb[0:rem, nfull * D:(nfull + 1) * D])

            def store_shard_pm(src_sb):
                """SBUF [128, NT*D] -> h_shard [NPAD, D] partition-major, contiguous."""
                nc.sync.dma_start(
                    out=h_shard.ap().rearrange("(p t) d -> p t d", t=NT),
                    in_=src_sb[:].rearrange("p (t d) -> p t d", d=D))

            # ================= embedding phase =================
            # h0 feature-major = t9.T @ acnt (per destination tile), then
            # transpose to node-major for the gather table.
            with tc.tile_pool(name="pse", bufs=2, space="PSUM") as pe_, \
                 tc.tile_pool(name="pst0", bufs=2, space="PSUM") as pt0:
                for nt in range(NT):
                    ps = pe_.tile([P, P], f32, tag="pse")
                    nc.tensor.matmul(out=ps[:], lhsT=t9_sb[:],
                                     rhs=acnt_sb[:, nt * P:(nt + 1) * P],
                                     start=True, stop=True)
                    nc.vector.tensor_scalar_add(
                        hTb[:, nt * P:(nt + 1) * P], ps[:], 0.0)
                    pst = pt0.tile([P, P], bf16, tag="pst0")
                    nc.tensor.transpose(
                        out=pst[:], in_=hTb[:, nt * P:(nt + 1) * P],
                        identity=idb_sb[:])
                    nc.scalar.activation(h_nm[:, nt * D:(nt + 1) * D],
                                         pst[:], ACT.Copy)
            store_shard_pm(h_nm)
            nc.gpsimd.collective_compute(
                "AllGather", mybir.AluOpType.bypass, replica_groups=RG,
                ins=[h_shard.ap().opt()], outs=[h_loc.ap().opt()])

            # ================= layers =================
            for l in range(L):
                w1_sb = load(wp, [D, 2 * D], f32, w1_in[l, :, :], tag="w1")
                w2a_sb = load(wp, [D, D], f32, w2_in[l, 0:D, :], tag="w2a")
                w2b_sb = load(wp, [D, D], f32, w2_in[l, D:2 * D, :], tag="w2b")
                t15_sb = load(wp, [16, D], f32, t15_in[l, :, :], tag="t15")

                b1a = prm_sb[:, l * 8 + 0:l * 8 + 1]
                b1b = prm_sb[:, l * 8 + 1:l * 8 + 2]
                b2 = prm_sb[:, l * 8 + 2:l * 8 + 3]
                gam = prm_sb[:, l * 8 + 3:l * 8 + 4]
                bet = prm_sb[:, l * 8 + 4:l * 8 + 5]

                # lazy chunked gathers + one-hot loads per stream; the
                # first chunks are small so the first PE chains unblock
                # quickly after the AllGather.
                def chunk_plan(et_s):
                    plan, t0 = [], 0
                    for sz in (8, 8):
                        if t0 >= et_s:
                            break
                        sz = min(sz, et_s - t0)
                        plan.append((t0, sz))
                        t0 += sz
                    while t0 < et_s:
                        sz = min(CH_TILES, et_s - t0)
                        plan.append((t0, sz))
                        t0 += sz
                    tile2chunk = []
                    for ci, (c0, sz) in enumerate(plan):
                        tile2chunk += [ci] * sz
                    return plan, tile2chunk

                chunks = {"lo": {}, "hi": {}}
                stream_cfg = {
                    "lo": (gi_lo, ohm_lo, chunk_plan(et_lo), 0),
                    "hi": (gi_hi, ohm_hi, chunk_plan(et_hi), VHALF),
                }

                def msg_slice(s, pos):
                    gi, ohm_d, (plan, t2c), row0 = stream_cfg[s]
                    ci = t2c[pos]
                    if ci not in chunks[s]:
                        t0, ntile = plan[ci]
                        nidx = ntile * P
                        g = mp.tile([P, CH_TILES * D], bf16, tag="msg")
                        nc.gpsimd.dma_gather(
                            out_ap=g[:, 0:ntile * D].rearrange("p (t e) -> p t e", e=D),
                            in_ap=h_loc[row0:NROWS, :] if row0 else h_loc[0:VHALF, :],
                            idxs_ap=gi[:, t0 * 8: t0 * 8 + nidx // 16],
                            num_idxs=nidx, num_idxs_reg=nidx, elem_size=D,
                            single_packet=False, queue_num=0)
                        ohc = op_.tile([P, CH_TILES * P], fp8, tag="ohc")
                        nc.scalar.dma_start(
                            out=ohc[:, 0:ntile * P],
                            in_=ohm_d[:, t0 * P:(t0 + ntile) * P])
                        chunks[s][ci] = (g, ohc, t0)
                    g, ohc, t0 = chunks[s][ci]
                    j = pos - t0
                    return g[:, j * D:(j + 1) * D], ohc[:, j * P:(j + 1) * P]

                with tc.tile_pool(name="psa", bufs=2, space="PSUM") as pa, \
                     tc.tile_pool(name="psh", bufs=2, space="PSUM") as ph, \
                     tc.tile_pool(name="pso", bufs=2, space="PSUM") as po:
                    for nt in range(NT):
                        psa = pa.tile([P, P], f32, tag="psa")
                        first = True
                        for s, karr, starts in (
                            ("lo", k_lo, start_lo),
                            ("hi", k_hi, start_hi),
                        ):
                            for jj in range(karr[nt]):
                                pos = starts[nt] + jj
                                m, oh = msg_slice(s, pos)
                                nc.tensor.matmul(out=psa[:], lhsT=m, rhs=oh,
                                                 start=first, stop=False)
                                first = False
                        nc.tensor.matmul(out=psa[:], lhsT=t15_sb[:],
                                         rhs=cnt_sb[:, nt * P:(nt + 1) * P],
                                         start=first, stop=True)

                        # aggr = psa + h_v (self-loop message), in bf16
                        aggr_b = ap_.tile([P, P], f32, tag="aggr")
                        nc.vector.tensor_tensor(
                            out=aggr_b[:], in0=psa[:],
                            in1=hTb[:, nt * P:(nt + 1) * P], op=ALU.add)

                        psh1 = ph.tile([P, P], f32, tag="psh")
                        nc.tensor.matmul(out=psh1[:], lhsT=w1_sb[:, 0:D],
                                         rhs=aggr_b[:], start=True, stop=True)
                        hidA = ap_.tile([P, P], f32, tag="hidA")
                        nc.scalar.activation(hidA[:], psh1[:], ACT.Relu, bias=b1a)
                        psh2 = ph.tile([P, P], f32, tag="psh")
                        nc.tensor.matmul(out=psh2[:], lhsT=w1_sb[:, D:2 * D],
                                         rhs=aggr_b[:], start=True, stop=True)
                        hidB = ap_.tile([P, P], f32, tag="hidB")
                        nc.scalar.activation(hidB[:], psh2[:], ACT.Relu, bias=b1b)

                        pso1 = po.tile([P, P], f32, tag="pso")
                        nc.tensor.matmul(out=pso1[:], lhsT=w2a_sb[:], rhs=hidA[:],
                                         start=True, stop=False)
                        nc.tensor.matmul(out=pso1[:], lhsT=w2b_sb[:], rhs=hidB[:],
                                         start=False, stop=True)
                        nc.vector.tensor_scalar_add(
                            outT[:, nt * P:(nt + 1) * P], pso1[:], b2)

                        # per-tile BN stat partials (valid node columns only)
                        c0 = nt * P
                        c1 = min((nt + 1) * P, NPC)
                        if c1 > c0:
                            nc.vector.tensor_reduce(
                                out=sumc_sb[:, nt:nt + 1],
                                in_=outT[:, c0:c1],
                                axis=mybir.AxisListType.X, op=ALU.add)
                            sq_scr = ap_.tile([P, P], f32, tag="sqscr")
                            nc.scalar.activation(
                                sq_scr[:, 0:c1 - c0], outT[:, c0:c1], ACT.Square,
                                accum_out=sqc_sb[:, nt:nt + 1])

                # ---- batch-norm statistics: combine tile partials ----
                nc.vector.tensor_reduce(
                    out=stats_sb[:, 0:1], in_=sumc_sb[:],
                    axis=mybir.AxisListType.X, op=ALU.add)
                nc.vector.tensor_reduce(
                    out=stats_sb[:, 1:2], in_=sqc_sb[:],
                    axis=mybir.AxisListType.X, op=ALU.add)
                nc.sync.dma_start(out=st_loc[:, :], in_=stats_sb[:])
                nc.gpsimd.collective_compute(
                    "AllReduce", ALU.add, replica_groups=RG,
                    ins=[st_loc.ap().opt()], outs=[st_glob.ap().opt()])
                nc.sync.dma_start(out=gstats_sb[:], in_=st_glob[:, :])

                nc.vector.tensor_scalar_mul(mean_c[:], gstats_sb[:, 0:1], inv_n)
                nc.vector.tensor_scalar_mul(ex2_c[:], gstats_sb[:, 1:2], inv_n)
                nc.scalar.activation(msq_c[:], mean_c[:], ACT.Square)
                nc.vector.tensor_tensor(var_c[:], ex2_c[:], msq_c[:], op=ALU.subtract)
                nc.vector.tensor_scalar_add(var_c[:], var_c[:], BN_EPS)
                nc.scalar.activation(sd_c[:], var_c[:], ACT.Sqrt)
                nc.vector.reciprocal(rstd_c[:], sd_c[:])
                nc.vector.tensor_tensor(k_c[:], gam, rstd_c[:], op=ALU.mult)
                nc.vector.tensor_tensor(tmp_c[:], mean_c[:], k_c[:], op=ALU.mult)
                nc.vector.tensor_tensor(c_c[:], bet, tmp_c[:], op=ALU.subtract)

                with tc.tile_pool(name="pst", bufs=2, space="PSUM") as pt:
                    if l < L - 1:
                        # h = relu(out*k + c) in bf16, transpose to node-major
                        nc.scalar.activation(hTb[:], outT[:], ACT.Relu,
                                             bias=c_c[:, 0:1], scale=k_c[:, 0:1])
                        for nt in range(NT):
                            pst = pt.tile([P, P], bf16, tag="pst")
                            nc.tensor.transpose(
                                out=pst[:], in_=hTb[:, nt * P:(nt + 1) * P],
                                identity=idb_sb[:])
                            nc.scalar.activation(h_nm[:, nt * D:(nt + 1) * D],
                                                 pst[:], ACT.Copy)
                        store_shard_pm(h_nm)
                        nc.gpsimd.collective_compute(
                            "AllGather", mybir.AluOpType.bypass, replica_groups=RG,
                            ins=[h_shard.ap().opt()], outs=[h_loc.ap().opt()])
                    else:
                        # final: out*k + c in place, transpose, store per tile
                        nc.vector.tensor_scalar(
                            out=outT[:], in0=outT[:],
                            scalar1=k_c[:, 0:1], scalar2=c_c[:, 0:1],
                            op0=ALU.mult, op1=ALU.add)
                        for nt in range(NT):
                            pst = pt.tile([P, P], f32, tag="pstf")
                            nc.tensor.transpose(
                                out=pst[:], in_=outT[:, nt * P:(nt + 1) * P],
                                identity=idf_sb[:])
                            o_t = ap_.tile([P, P], f32, tag="otile")
                            nc.scalar.activation(o_t[:], pst[:], ACT.Copy)
                            r0 = nt * P
                            r1 = min((nt + 1) * P, NPC)
                            if r1 > r0:
                                nc.sync.dma_start(
                                    out=out_ext[r0:r1, :],
                                    in_=o_t[0:r1 - r0, :])

    # Align each gather's SWDGE queue with the DMASW semaphore lane Tile
    # assigned it (lane k <-> queue k % NQ), so no semaphore is shared by
    # two queues (completion order within a lane must match issue order).
    from concourse.tile_scheduler import PROC_NAME_TO_IDX
    dmasw0 = PROC_NAME_TO_IDX["DMASW0"]
    for inst in nc.inst_map.values():
        if isinstance(inst, mybir.InstDMAGatherAnt):
            proc = inst.bass_scheduled_proc
            assert proc is not None and dmasw0 <= proc < dmasw0 + 8, (
                f"gather {inst.name} not on a DMASW lane: {proc}")
            inst.queue_num = (proc - dmasw0) % NQ

    nc.compile()
    return nc


_CACHE = {}


def _make_in_maps(arr, atom_emb0, atom_emb1, edge_emb0, edge_emb1,
                  W1, b1, W2, b2, gamma, beta):
    import ml_dtypes
    # ---- parameter tables (host float prep limited to tiny tables) ----
    ae0 = np.asarray(atom_emb0, np.float32)
    ae1 = np.asarray(atom_emb1, np.float32)
    ee0 = np.asarray(edge_emb0, np.float32)
    ee1 = np.asarray(edge_emb1, np.float32)
    t9 = np.zeros((16, D), np.float32)
    t9[:9] = (ae0[:3, None, :] + ae1[None, :3, :]).reshape(9, D)
    t15 = np.zeros((L, 16, D), np.float32)
    for l in range(L):
        t15[l, :15] = (ee0[l][:, None, :] + ee1[l][None, :, :]).reshape(15, D)

    W1 = np.asarray(W1, np.float32)
    W2 = np.asarray(W2, np.float32)
    b1 = np.asarray(b1, np.float32)
    b2 = np.asarray(b2, np.float32)
    gamma = np.asarray(gamma, np.float32)
    beta = np.asarray(beta, np.float32)
    prmT = np.zeros((P, 8 * L), np.float32)
    for l in range(L):
        prmT[:, l * 8 + 0] = b1[l, 0:D]
        prmT[:, l * 8 + 1] = b1[l, D:2 * D]
        prmT[:, l * 8 + 2] = b2[l]
        prmT[:, l * 8 + 3] = gamma[l]
        prmT[:, l * 8 + 4] = beta[l]

    ident = np.eye(P, dtype=np.float32)
    bf = ml_dtypes.bfloat16

    in_maps = []
    for c in range(NCORES):
        in_maps.append({
            "gidx_lo": _wrap_idx_cols(arr["gidx_lo"][c]),
            "gidx_hi": _wrap_idx_cols(arr["gidx_hi"][c]),
            "ohm_lo": arr["ohm_lo"][c].astype(ml_dtypes.float8_e4m3),
            "ohm_hi": arr["ohm_hi"][c].astype(ml_dtypes.float8_e4m3),
            "cnt": arr["cnt"][c],
            "acnt": arr["acnt"][c].astype(bf),
            "prmT": prmT,
            "w1": W1,
            "w2": W2,
            "t9": t9.astype(bf),
            "t15": t15,
            "identf": ident,
            "identb": ident.astype(bf),
        })
    return in_maps


def kernel(x, edge_index, edge_attr, atom_emb0, atom_emb1,
           edge_emb0, edge_emb1, W1, b1, W2, b2, gamma, beta):
    from concourse.bass_utils import run_bass_kernel_spmd

    sched, arr = _preprocess(x, edge_index, edge_attr)
    key = (tuple(sched["k_lo"]), tuple(sched["k_hi"]))
    if key not in _CACHE:
        _CACHE[key] = _build(sched)
    nc = _CACHE[key]

    in_maps = _make_in_maps(arr, atom_emb0, atom_emb1, edge_emb0, edge_emb1,
                            W1, b1, W2, b2, gamma, beta)
    res = run_bass_kernel_spmd(nc, in_maps, core_ids=list(range(NCORES)))
    out = np.concatenate([res.results[c]["out"] for c in range(NCORES)], axis=0)
    return out.astype(np.float32)
